# revision 1
# baseline (speedup 1.0000x reference)
"""GCN layer (gather -> mean-aggregate -> linear -> relu) on 8 TRN2 NeuronCores.

Strategy:
- Nodes (and output rows) are sharded by destination across the 8 cores
  (12500 dsts each); edges are partitioned by destination core. h and the
  64x64 weight are replicated to every core.
- Reorder: out = relu(mask * (mean_agg(h) @ W.T + b)). The linear layer
  commutes with the mean aggregation, so each core gathers raw h rows for
  its edges, segment-sums them per dst (via one-hot selection matmuls that
  also accumulate the degree in a 65th column), then applies the projection
  per 128-dst block.
- The gather uses dma_gather (SWDGE) from HBM with int16 indices; h is
  split into 4 row-groups of 32768 so indices fit int16. Edges are grouped
  (dst-block, src-group) with all-core-uniform static capacities (idx-0
  pads, killed in the one-hot since their dst slot is -1).
"""

import numpy as np
from contextlib import ExitStack

N_NODES = 100000
N_EDGES = 1600000
D = 64
NCORES = 8
NPC = N_NODES // NCORES          # dsts per core
NB = (NPC + 127) // 128          # dst blocks per core
GS = 32768                       # src group size (int16 index range)
NG = (N_NODES + GS - 1) // GS    # src groups
MAX_IDX_PER_INSTR = 1024


def _round16(x):
    return (x + 15) & ~15


def _host_partition(edge_src, edge_dst):
    """Partition+group edges; build per-core idx16/dstval arrays and the
    shared static instruction plan."""
    core = edge_dst // NPC
    per_core = []
    counts = np.zeros((NCORES, NB, NG), np.int64)
    for c in range(NCORES):
        m = np.nonzero(core == c)[0]
        src_c = edge_src[m]
        dst_c = edge_dst[m] - c * NPC
        blk = dst_c >> 7
        grp = src_c >> 15
        order = np.lexsort((grp, blk))
        src_c = src_c[order]
        dst_c = dst_c[order]
        blk = blk[order]
        grp = grp[order]
        cell = blk * NG + grp
        cnt = np.bincount(cell, minlength=NB * NG).reshape(NB, NG)
        counts[c] = cnt
        per_core.append((src_c, dst_c))

    caps = counts.max(axis=0)
    # static plan: list of (B, g, n_idxs); n multiple of 16, <= 1024
    plan = []
    for B in range(NB):
        for g in range(NG):
            cap = _round16(int(caps[B, g]))
            while cap > 0:
                n = min(cap, MAX_IDX_PER_INSTR)
                plan.append((B, g, n))
                cap -= n
    # per-instr chunk counts and offsets
    instr_chunks = [(n + 127) // 128 for (_, _, n) in plan]
    tot_chunks = sum(instr_chunks)
    idx_w = sum(n // 16 for (_, _, n) in plan)

    idx_arrs = []
    dv_arrs = []
    deg_arrs = []
    for c in range(NCORES):
        src_c, dst_c = per_core[c]
        deg = np.bincount(dst_c, minlength=NB * 128).astype(np.float32)
        rdeg = 1.0 / np.maximum(deg, 1.0)
        ind = np.minimum(deg, 1.0)
        deg_arrs.append((rdeg.reshape(NB, 128).T.copy(), ind.reshape(NB, 128).T.copy()))
        # start offset of each (B, g) cell in the sorted edge arrays
        cnt = counts[c]
        cell_sizes = cnt.reshape(-1)
        cell_starts = np.zeros(NB * NG + 1, np.int64)
        np.cumsum(cell_sizes, out=cell_starts[1:])
        consumed = np.zeros(NB * NG, np.int64)

        idx16 = np.zeros((128, idx_w), np.int16)
        dv = np.full((128, tot_chunks), -1.0, np.float32)
        wofs = 0
        chofs = 0
        for (B, g, n) in plan:
            ci = B * NG + g
            k = int(min(cnt[B, g] - consumed[ci], n))
            s0 = int(cell_starts[ci] + consumed[ci])
            consumed[ci] += k
            flat = np.zeros(n, np.int16)
            if k > 0:
                flat[:k] = (src_c[s0:s0 + k] & (GS - 1)).astype(np.int16)
                e = np.arange(k)
                dv[e % 128, chofs + e // 128] = (dst_c[s0:s0 + k] & 127).astype(np.float32)
            w = n // 16
            idx16[:, wofs:wofs + w] = np.tile(flat.reshape(w, 16).T, (8, 1))
            wofs += w
            chofs += (n + 127) // 128
        idx_arrs.append(idx16)
        dv_arrs.append(dv)
    return plan, instr_chunks, tot_chunks, idx_w, idx_arrs, dv_arrs, deg_arrs


def _build_nc(plan, instr_chunks, tot_chunks, idx_w):
    import concourse.bacc as bacc
    import concourse.mybir as mybir
    from concourse.library_config import mlp
    from concourse._compat import get_trn_type

    f32 = mybir.dt.float32
    i16 = mybir.dt.int16
    glens = [min(GS, N_NODES - g * GS) for g in range(NG)]

    # per-instr bookkeeping
    ninstr = len(plan)
    end_chunk = np.cumsum(instr_chunks)  # chunks consumed when instr i done
    # chunks per block
    blk_chunks = np.zeros(NB, np.int64)
    for (B, g, n), ch in zip(plan, instr_chunks):
        blk_chunks[B] += ch
    blk_end = np.cumsum(blk_chunks)      # pe_s value when block B mms done

    nc = bacc.Bacc(get_trn_type() or "TRN2", debug=True, num_swdge_queues=4)
    h_d = nc.declare_dram_parameter("h", [N_NODES, D], f32, isOutput=False)
    idx_d = nc.declare_dram_parameter("idx", [128, idx_w], i16, isOutput=False)
    dv_d = nc.declare_dram_parameter("dv", [128, tot_chunks], f32, isOutput=False)
    cst_d = nc.declare_dram_parameter("cst", [128, 257], f32, isOutput=False)
    wa_d = nc.declare_dram_parameter("wa", [65, D], f32, isOutput=False)
    rdeg_d = nc.declare_dram_parameter("rdeg", [128, NB], f32, isOutput=False)
    ind_d = nc.declare_dram_parameter("ind", [128, NB], f32, isOutput=False)
    out_d = nc.declare_dram_parameter("out", [NB * 128, D], f32, isOutput=True)

    with ExitStack() as st:
        e = st.enter_context
        idx_sb = e(nc.sbuf_tensor("idx_sb", [128, idx_w], i16))
        dv_sb = e(nc.sbuf_tensor("dv_sb", [128, tot_chunks], f32))
        cst_sb = e(nc.sbuf_tensor("cst_sb", [128, 257], f32))
        wa_sb = e(nc.sbuf_tensor("wa_sb", [65, D], f32))
        rdeg_sb = e(nc.sbuf_tensor("rdeg_sb", [128, NB], f32))
        ind_sb = e(nc.sbuf_tensor("ind_sb", [128, NB], f32))
        gbuf = [e(nc.sbuf_tensor(f"gbuf{i}", [128, 8 * D], f32)) for i in range(4)]
        sel = [e(nc.sbuf_tensor(f"sel{i}", [128, 128], f32)) for i in range(8)]
        agg = [e(nc.sbuf_tensor(f"agg{i}", [128, 65], f32)) for i in range(2)]
        aggT = [e(nc.sbuf_tensor(f"aggT{i}", [65, 128], f32)) for i in range(2)]
        ysb = [e(nc.sbuf_tensor(f"ysb{i}", [64, 128], f32)) for i in range(2)]
        otile = [e(nc.sbuf_tensor(f"otile{i}", [128, D], f32)) for i in range(2)]
        tmpm = [e(nc.sbuf_tensor(f"tmpm{i}", [128, 1], f32)) for i in range(2)]
        tmpr = [e(nc.sbuf_tensor(f"tmpr{i}", [128, 1], f32)) for i in range(2)]

        acc = [e(nc.psum_tensor(f"acc{i}", [128, 65], f32)) for i in range(2)]
        pt1 = [e(nc.psum_tensor("pt1", [65, 128], f32))] * 2
        pmw = [e(nc.psum_tensor("pmw", [64, 128], f32))] * 2
        pt2 = [e(nc.psum_tensor("pt2", [128, D], f32))] * 2

        in_s = e(nc.semaphore("in_s"))
        g_s = [e(nc.semaphore(f"g_s{i}")) for i in range(4)]
        pe_s = e(nc.semaphore("pe_s"))
        sel_s = e(nc.semaphore("sel_s"))
        dep_s = e(nc.semaphore("dep_s"))
        pt1_s = e(nc.semaphore("pt1_s"))
        dt1_s = e(nc.semaphore("dt1_s"))
        pmw_s = e(nc.semaphore("pmw_s"))
        act_s = e(nc.semaphore("act_s"))
        pt2_s = e(nc.semaphore("pt2_s"))
        dvo_s = e(nc.semaphore("dvo_s"))
        out_s = e(nc.semaphore("out_s"))
        ms_s = e(nc.semaphore("ms_s"))
        block = e(nc.Block())

        iota_ap = lambda: cst_sb[:, 0:128]
        ident_ap = lambda: cst_sb[:, 128:256]
        ones_ap = lambda: cst_sb[:, 256:257]

        # instr index ranges per block (for PE wait bookkeeping)
        instr_of_chunk = []
        first_chunk_of_instr = []
        kc = 0
        for i, ch in enumerate(instr_chunks):
            first_chunk_of_instr.append(kc)
            for j in range(ch):
                instr_of_chunk.append((i, j))
            kc += ch

        @block.gpsimd
        def _(eng):
            eng.load_library(mlp)
            eng.dma_start(out=idx_sb[:], in_=idx_d[:]).then_inc(in_s, 16)
            eng.dma_start(out=dv_sb[:], in_=dv_d[:]).then_inc(in_s, 16)
            eng.dma_start(out=cst_sb[:], in_=cst_d[:]).then_inc(in_s, 16)
            eng.dma_start(out=wa_sb[:], in_=wa_d[:]).then_inc(in_s, 16)
            eng.dma_start(out=rdeg_sb[:], in_=rdeg_d[:]).then_inc(in_s, 16)
            eng.dma_start(out=ind_sb[:], in_=ind_d[:]).then_inc(in_s, 16)
            eng.wait_ge(in_s, 96)
            for bb in range(4):
                eng.memset(gbuf[bb][:], 0.0).then_inc(ms_s, 1)
            eng.wait_ge(ms_s, 4)
            wofs = 0
            for i, (B, g, n) in enumerate(plan):
                if i >= 4:
                    eng.wait_ge(pe_s, int(end_chunk[i - 4]))
                kb = (n + 127) // 128
                eng.dma_gather(
                    out_ap=gbuf[i % 4][:, : kb * D].rearrange(
                        "p (k d) -> p k d", d=D
                    ),
                    in_ap=h_d[g * GS : g * GS + glens[g], :],
                    idxs_ap=idx_sb[:, wofs : wofs + n // 16],
                    num_idxs=n,
                    num_idxs_reg=n,
                    elem_size=D,
                    queue_num=i % 4,
                ).then_inc(g_s[i % 4], 16)
                wofs += n // 16

        @block.tensor
        def _(eng):
            eng.wait_ge(in_s, 96)

            def pe_ep(Bp):
                p = Bp % 2
                eng.wait_ge(dep_s, Bp + 1)
                if Bp >= 1:
                    eng.wait_ge(dt1_s, Bp)
                eng.matmul(
                    out=pt1[p][:], lhsT=agg[p][:], rhs=ident_ap(),
                    is_transpose=True,
                ).then_inc(pt1_s, 1)
                eng.wait_ge(dt1_s, Bp + 1)
                if Bp >= 1:
                    eng.wait_ge(act_s, Bp)
                eng.matmul(
                    out=pmw[p][:], lhsT=wa_sb[:], rhs=aggT[p][:],
                    start=True, stop=True,
                ).then_inc(pmw_s, 1)
                eng.wait_ge(act_s, Bp + 1)
                if Bp >= 1:
                    eng.wait_ge(dvo_s, Bp)
                eng.matmul(
                    out=pt2[p][:], lhsT=ysb[p][:], rhs=ident_ap()[:64, :64],
                    is_transpose=True,
                ).then_inc(pt2_s, 1)

            kchunk = 0
            for B in range(NB):
                cb = int(blk_chunks[B])
                for j in range(cb):
                    i, jin = instr_of_chunk[kchunk]
                    if jin == 0:
                        eng.wait_ge(g_s[i % 4], 16 * (i // 4 + 1))
                    eng.wait_ge(sel_s, kchunk + 1)
                    if j == 0 and B >= 2:
                        eng.wait_ge(dep_s, B - 1)
                    first = j == 0
                    last = j == cb - 1
                    eng.matmul(
                        out=acc[B % 2][:, 0:64],
                        lhsT=sel[kchunk % 8][:],
                        rhs=gbuf[i % 4][:, jin * D : (jin + 1) * D],
                        start=first, stop=last,
                    ).then_inc(pe_s, 1)
                    kchunk += 1
                if B >= 1:
                    pe_ep(B - 1)
            pe_ep(NB - 1)

        @block.vector
        def _(eng):
            import concourse.mybir as mb
            eng.wait_ge(in_s, 96)

            def dve_ep(Bp):
                p = Bp % 2
                eng.wait_ge(pe_s, int(blk_end[Bp]))
                if Bp >= 2:
                    eng.wait_ge(pt1_s, Bp - 1)
                eng.tensor_copy(out=agg[p][:, 64:65], in_=ind_sb[:, Bp : Bp + 1])
                eng.tensor_scalar(
                    out=agg[p][:, 0:64], in0=acc[p][:, 0:64],
                    scalar1=rdeg_sb[:, Bp : Bp + 1], scalar2=None,
                    op0=mb.AluOpType.mult,
                ).then_inc(dep_s, 1)
                eng.wait_ge(pt1_s, Bp + 1)
                if Bp >= 2:
                    eng.wait_ge(pmw_s, Bp - 1)
                eng.tensor_copy(out=aggT[p][:], in_=pt1[p][:]).then_inc(dt1_s, 1)
                eng.wait_ge(pt2_s, Bp + 1)
                if Bp >= 2:
                    eng.wait_ge(out_s, 16 * (Bp - 1))
                eng.tensor_copy(out=otile[p][:], in_=pt2[p][:]).then_inc(dvo_s, 1)

            kchunk = 0
            for B in range(NB):
                cb = int(blk_chunks[B])
                for j in range(cb):
                    if kchunk >= 8:
                        eng.wait_ge(pe_s, kchunk - 7)
                    eng.tensor_tensor(
                        out=sel[kchunk % 8][:],
                        in0=dv_sb[:, kchunk : kchunk + 1].to_broadcast([128, 128]),
                        in1=iota_ap(),
                        op=mb.AluOpType.is_equal,
                    ).then_inc(sel_s, 1)
                    kchunk += 1
                if B >= 1:
                    dve_ep(B - 1)
            dve_ep(NB - 1)

        @block.scalar
        def _(eng):
            import concourse.mybir as mb
            for B in range(NB):
                eng.wait_ge(pmw_s, B + 1)
                if B >= 2:
                    eng.wait_ge(pt2_s, B - 1)
                eng.activation(
                    out=ysb[B % 2][:], in_=pmw[B % 2][:],
                    func=mb.ActivationFunctionType.Relu,
                ).then_inc(act_s, 1)

        @block.sync
        def _(eng):
            for B in range(NB):
                eng.wait_ge(dvo_s, B + 1)
                eng.dma_start(
                    out=out_d[B * 128 : (B + 1) * 128, :], in_=otile[B % 2][:]
                ).then_inc(out_s, 16)
            eng.wait_ge(out_s, 16 * NB)

    nc.compile()
    return nc


def _host_inputs(h, W, b, idx_arrs, dv_arrs, deg_arrs):
    cst = np.zeros((128, 257), np.float32)
    cst[:, 0:128] = np.arange(128, dtype=np.float32)[None, :]
    cst[:, 128:256] = np.eye(128, dtype=np.float32)
    cst[:, 256] = 1.0
    wa = np.concatenate([W.T.astype(np.float32), b.astype(np.float32)[None, :]], axis=0)
    in_maps = []
    for c in range(NCORES):
        in_maps.append({
            "h": np.ascontiguousarray(h.astype(np.float32)),
            "idx": idx_arrs[c],
            "dv": dv_arrs[c],
            "cst": cst,
            "wa": wa,
            "rdeg": deg_arrs[c][0],
            "ind": deg_arrs[c][1],
        })
    return in_maps


def kernel(h, edge_src, edge_dst, W, b):
    h = np.asarray(h, np.float32)
    edge_src = np.asarray(edge_src, np.int32)
    edge_dst = np.asarray(edge_dst, np.int32)
    W = np.asarray(W, np.float32)
    b = np.asarray(b, np.float32)

    from concourse.bass_utils import run_bass_kernel_spmd

    plan, instr_chunks, tot_chunks, idx_w, idx_arrs, dv_arrs, deg_arrs = _host_partition(
        edge_src, edge_dst
    )
    nc = _build_nc(plan, instr_chunks, tot_chunks, idx_w)
    in_maps = _host_inputs(h, W, b, idx_arrs, dv_arrs, deg_arrs)
    res = run_bass_kernel_spmd(nc, in_maps, list(range(NCORES)))
    out = np.concatenate(
        [res.results[c]["out"][:NPC] for c in range(NCORES)], axis=0
    )
    return out.astype(np.float32)



# revision 12
# speedup vs baseline: 1.0056x; 1.0056x over previous
"""GCN layer (gather -> mean-aggregate -> linear -> relu) on 8 TRN2 NeuronCores.

Strategy:
- Nodes (and output rows) are sharded by destination across the 8 cores
  (12500 dsts each); edges are partitioned by destination core. h and the
  64x64 weight are replicated to every core.
- Reorder: out = relu(mask * (mean_agg(h) @ W.T + b)). The linear layer
  commutes with the mean aggregation, so each core gathers raw h rows for
  its edges, segment-sums them per dst (via one-hot selection matmuls that
  also accumulate the degree in a 65th column), then applies the projection
  per 128-dst block.
- The gather uses dma_gather (SWDGE) from HBM with int16 indices; h is
  split into 4 row-groups of 32768 so indices fit int16. Edges are grouped
  (dst-block, src-group) with all-core-uniform static capacities (idx-0
  pads, killed in the one-hot since their dst slot is -1).
"""

import numpy as np
from contextlib import ExitStack

N_NODES = 100000
N_EDGES = 1600000
D = 64
NCORES = 8
NPC = N_NODES // NCORES          # dsts per core
NB = (NPC + 127) // 128          # dst blocks per core
GS = 32768                       # src group size (int16 index range)
NG = (N_NODES + GS - 1) // GS    # src groups
MAX_IDX_PER_INSTR = 1024


def _round16(x):
    return (x + 15) & ~15


def _host_partition(edge_src, edge_dst):
    """Partition+group edges; build per-core idx16/dstval arrays and the
    shared static instruction plan."""
    core = edge_dst // NPC
    per_core = []
    counts = np.zeros((NCORES, NB, NG), np.int64)
    for c in range(NCORES):
        m = np.nonzero(core == c)[0]
        src_c = edge_src[m]
        dst_c = edge_dst[m] - c * NPC
        blk = dst_c >> 7
        grp = src_c >> 15
        order = np.lexsort((grp, blk))
        src_c = src_c[order]
        dst_c = dst_c[order]
        blk = blk[order]
        grp = grp[order]
        cell = blk * NG + grp
        cnt = np.bincount(cell, minlength=NB * NG).reshape(NB, NG)
        counts[c] = cnt
        per_core.append((src_c, dst_c))

    caps = counts.max(axis=0)
    # static plan: list of (B, g, n_idxs); n multiple of 16, <= 1024
    plan = []
    for B in range(NB):
        for g in range(NG):
            cap = _round16(int(caps[B, g]))
            while cap > 0:
                n = min(cap, MAX_IDX_PER_INSTR)
                plan.append((B, g, n))
                cap -= n
    # per-instr chunk counts and offsets
    instr_chunks = [(n + 127) // 128 for (_, _, n) in plan]
    tot_chunks = sum(instr_chunks)
    idx_w = sum(n // 16 for (_, _, n) in plan)

    idx_arrs = []
    dv_arrs = []
    deg_arrs = []
    for c in range(NCORES):
        src_c, dst_c = per_core[c]
        deg = np.bincount(dst_c, minlength=NB * 128).astype(np.float32)
        rdeg = 1.0 / np.maximum(deg, 1.0)
        ind = np.minimum(deg, 1.0)
        deg_arrs.append((rdeg.reshape(NB, 128).T.copy(), ind.reshape(NB, 128).T.copy()))
        # start offset of each (B, g) cell in the sorted edge arrays
        cnt = counts[c]
        cell_sizes = cnt.reshape(-1)
        cell_starts = np.zeros(NB * NG + 1, np.int64)
        np.cumsum(cell_sizes, out=cell_starts[1:])
        consumed = np.zeros(NB * NG, np.int64)

        idx16 = np.zeros((128, idx_w), np.int16)
        dv = np.full((128, tot_chunks), -1.0, np.float32)
        wofs = 0
        chofs = 0
        for (B, g, n) in plan:
            ci = B * NG + g
            k = int(min(cnt[B, g] - consumed[ci], n))
            s0 = int(cell_starts[ci] + consumed[ci])
            consumed[ci] += k
            flat = np.zeros(n, np.int16)
            if k > 0:
                flat[:k] = (src_c[s0:s0 + k] & (GS - 1)).astype(np.int16)
                e = np.arange(k)
                dv[e % 128, chofs + e // 128] = (dst_c[s0:s0 + k] & 127).astype(np.float32)
            w = n // 16
            idx16[:, wofs:wofs + w] = np.tile(flat.reshape(w, 16).T, (8, 1))
            wofs += w
            chofs += (n + 127) // 128
        idx_arrs.append(idx16)
        dv_arrs.append(dv)
    return plan, instr_chunks, tot_chunks, idx_w, idx_arrs, dv_arrs, deg_arrs


def _build_nc(plan, instr_chunks, tot_chunks, idx_w):
    import concourse.bacc as bacc
    import concourse.mybir as mybir
    from concourse.library_config import mlp
    from concourse._compat import get_trn_type

    f32 = mybir.dt.float32
    i16 = mybir.dt.int16
    glens = [min(GS, N_NODES - g * GS) for g in range(NG)]

    # per-instr bookkeeping
    ninstr = len(plan)
    end_chunk = np.cumsum(instr_chunks)  # chunks consumed when instr i done
    # chunks per block
    blk_chunks = np.zeros(NB, np.int64)
    for (B, g, n), ch in zip(plan, instr_chunks):
        blk_chunks[B] += ch
    blk_end = np.cumsum(blk_chunks)      # pe_s value when block B mms done

    nc = bacc.Bacc(get_trn_type() or "TRN2", debug=True, num_swdge_queues=4)
    h_d = nc.declare_dram_parameter("h", [N_NODES, D], f32, isOutput=False)
    idx_d = nc.declare_dram_parameter("idx", [128, idx_w], i16, isOutput=False)
    dv_d = nc.declare_dram_parameter("dv", [128, tot_chunks], f32, isOutput=False)
    cst_d = nc.declare_dram_parameter("cst", [128, 257], f32, isOutput=False)
    wa_d = nc.declare_dram_parameter("wa", [65, D], f32, isOutput=False)
    rdeg_d = nc.declare_dram_parameter("rdeg", [128, NB], f32, isOutput=False)
    ind_d = nc.declare_dram_parameter("ind", [128, NB], f32, isOutput=False)
    out_d = nc.declare_dram_parameter("out", [NB * 128, D], f32, isOutput=True)

    with ExitStack() as st:
        e = st.enter_context
        idx_sb = e(nc.sbuf_tensor("idx_sb", [128, idx_w], i16))
        dv_sb = e(nc.sbuf_tensor("dv_sb", [128, tot_chunks], f32))
        cst_sb = e(nc.sbuf_tensor("cst_sb", [128, 257], f32))
        wa_sb = e(nc.sbuf_tensor("wa_sb", [65, D], f32))
        rdeg_sb = e(nc.sbuf_tensor("rdeg_sb", [128, NB], f32))
        ind_sb = e(nc.sbuf_tensor("ind_sb", [128, NB], f32))
        gbuf = [e(nc.sbuf_tensor(f"gbuf{i}", [128, 8 * D], f32)) for i in range(4)]
        sel = [e(nc.sbuf_tensor(f"sel{i}", [128, 128], f32)) for i in range(8)]
        agg = [e(nc.sbuf_tensor(f"agg{i}", [128, 65], f32)) for i in range(2)]
        aggT = [e(nc.sbuf_tensor(f"aggT{i}", [65, 128], f32)) for i in range(2)]
        ysb = [e(nc.sbuf_tensor(f"ysb{i}", [64, 128], f32)) for i in range(2)]
        otile = [e(nc.sbuf_tensor(f"otile{i}", [128, D], f32)) for i in range(2)]
        tmpm = [e(nc.sbuf_tensor(f"tmpm{i}", [128, 1], f32)) for i in range(2)]
        tmpr = [e(nc.sbuf_tensor(f"tmpr{i}", [128, 1], f32)) for i in range(2)]

        acc = [e(nc.psum_tensor(f"acc{i}", [128, 65], f32)) for i in range(2)]
        pt1 = [e(nc.psum_tensor("pt1", [65, 128], f32))] * 2
        pmw = [e(nc.psum_tensor("pmw", [64, 128], f32))] * 2
        pt2 = [e(nc.psum_tensor("pt2", [128, D], f32))] * 2

        in_s = e(nc.semaphore("in_s"))
        g_s = [e(nc.semaphore(f"g_s{i}")) for i in range(4)]
        pe_s = e(nc.semaphore("pe_s"))
        sel_s = e(nc.semaphore("sel_s"))
        dep_s = e(nc.semaphore("dep_s"))
        pt1_s = e(nc.semaphore("pt1_s"))
        dt1_s = e(nc.semaphore("dt1_s"))
        pmw_s = e(nc.semaphore("pmw_s"))
        act_s = e(nc.semaphore("act_s"))
        pt2_s = e(nc.semaphore("pt2_s"))
        dvo_s = e(nc.semaphore("dvo_s"))
        out_s = e(nc.semaphore("out_s"))
        ms_s = e(nc.semaphore("ms_s"))
        block = e(nc.Block())

        iota_ap = lambda: cst_sb[:, 0:128]
        ident_ap = lambda: cst_sb[:, 128:256]
        ones_ap = lambda: cst_sb[:, 256:257]

        # instr index ranges per block (for PE wait bookkeeping)
        instr_of_chunk = []
        first_chunk_of_instr = []
        kc = 0
        for i, ch in enumerate(instr_chunks):
            first_chunk_of_instr.append(kc)
            for j in range(ch):
                instr_of_chunk.append((i, j))
            kc += ch

        @block.gpsimd
        def _(eng):
            eng.load_library(mlp)
            eng.dma_start(out=idx_sb[:], in_=idx_d[:]).then_inc(in_s, 16)
            eng.dma_start(out=dv_sb[:], in_=dv_d[:]).then_inc(in_s, 16)
            eng.dma_start(out=cst_sb[:], in_=cst_d[:]).then_inc(in_s, 16)
            eng.dma_start(out=wa_sb[:], in_=wa_d[:]).then_inc(in_s, 16)
            eng.dma_start(out=rdeg_sb[:], in_=rdeg_d[:]).then_inc(in_s, 16)
            eng.dma_start(out=ind_sb[:], in_=ind_d[:]).then_inc(in_s, 16)
            eng.wait_ge(in_s, 96)
            for bb in range(4):
                eng.memset(gbuf[bb][:], 0.0).then_inc(ms_s, 1)
            eng.wait_ge(ms_s, 4)
            wofs = 0
            for i, (B, g, n) in enumerate(plan):
                if i >= 4:
                    eng.wait_ge(pe_s, int(end_chunk[i - 4]))
                kb = (n + 127) // 128
                eng.dma_gather(
                    out_ap=gbuf[i % 4][:, : kb * D].rearrange(
                        "p (k d) -> p k d", d=D
                    ),
                    in_ap=h_d[g * GS : g * GS + glens[g], :],
                    idxs_ap=idx_sb[:, wofs : wofs + n // 16],
                    num_idxs=n,
                    num_idxs_reg=n,
                    elem_size=D,
                    queue_num=i % 4,
                ).then_inc(g_s[i % 4], 16)
                wofs += n // 16

        @block.tensor
        def _(eng):
            eng.wait_ge(in_s, 96)

            def pe_ep(Bp):
                p = Bp % 2
                eng.wait_ge(dep_s, Bp + 1)
                if Bp >= 1:
                    eng.wait_ge(dt1_s, Bp)
                eng.matmul(
                    out=pt1[p][:], lhsT=agg[p][:], rhs=ident_ap(),
                    is_transpose=True,
                ).then_inc(pt1_s, 1)
                eng.wait_ge(dt1_s, Bp + 1)
                if Bp >= 1:
                    eng.wait_ge(act_s, Bp)
                eng.matmul(
                    out=pmw[p][:], lhsT=wa_sb[:], rhs=aggT[p][:],
                    start=True, stop=True,
                ).then_inc(pmw_s, 1)
                eng.wait_ge(act_s, Bp + 1)
                if Bp >= 1:
                    eng.wait_ge(dvo_s, Bp)
                eng.matmul(
                    out=pt2[p][:], lhsT=ysb[p][:], rhs=ident_ap()[:64, :64],
                    is_transpose=True,
                ).then_inc(pt2_s, 1)

            kchunk = 0
            for B in range(NB):
                cb = int(blk_chunks[B])
                for j in range(cb):
                    i, jin = instr_of_chunk[kchunk]
                    if jin == 0:
                        eng.wait_ge(g_s[i % 4], 16 * (i // 4 + 1))
                    eng.wait_ge(sel_s, kchunk + 1)
                    if j == 0 and B >= 2:
                        eng.wait_ge(dep_s, B - 1)
                    first = j == 0
                    last = j == cb - 1
                    eng.matmul(
                        out=acc[B % 2][:, 0:64],
                        lhsT=sel[kchunk % 8][:],
                        rhs=gbuf[i % 4][:, jin * D : (jin + 1) * D],
                        start=first, stop=last,
                    ).then_inc(pe_s, 1)
                    kchunk += 1
                if B >= 1:
                    pe_ep(B - 1)
            pe_ep(NB - 1)

        @block.vector
        def _(eng):
            import concourse.mybir as mb
            eng.wait_ge(in_s, 96)

            def dve_ep(Bp):
                p = Bp % 2
                eng.wait_ge(pe_s, int(blk_end[Bp]))
                if Bp >= 2:
                    eng.wait_ge(pt1_s, Bp - 1)
                eng.tensor_copy(out=agg[p][:, 64:65], in_=ind_sb[:, Bp : Bp + 1])
                eng.tensor_scalar(
                    out=agg[p][:, 0:64], in0=acc[p][:, 0:64],
                    scalar1=rdeg_sb[:, Bp : Bp + 1], scalar2=None,
                    op0=mb.AluOpType.mult,
                ).then_inc(dep_s, 1)
                eng.wait_ge(pt1_s, Bp + 1)
                if Bp >= 2:
                    eng.wait_ge(pmw_s, Bp - 1)
                eng.tensor_copy(out=aggT[p][:], in_=pt1[p][:]).then_inc(dt1_s, 1)
                eng.wait_ge(pt2_s, Bp + 1)
                if Bp >= 2:
                    eng.wait_ge(out_s, 16 * (Bp - 1))
                eng.tensor_copy(out=otile[p][:], in_=pt2[p][:]).then_inc(dvo_s, 1)

            kchunk = 0
            for B in range(NB):
                cb = int(blk_chunks[B])
                for j in range(cb):
                    if kchunk >= 8:
                        eng.wait_ge(pe_s, kchunk - 7)
                    eng.tensor_tensor(
                        out=sel[kchunk % 8][:],
                        in0=dv_sb[:, kchunk : kchunk + 1].to_broadcast([128, 128]),
                        in1=iota_ap(),
                        op=mb.AluOpType.is_equal,
                    ).then_inc(sel_s, 1)
                    kchunk += 1
                if B >= 1:
                    dve_ep(B - 1)
            dve_ep(NB - 1)

        @block.scalar
        def _(eng):
            import concourse.mybir as mb
            for B in range(NB):
                eng.wait_ge(pmw_s, B + 1)
                if B >= 2:
                    eng.wait_ge(pt2_s, B - 1)
                eng.activation(
                    out=ysb[B % 2][:], in_=pmw[B % 2][:],
                    func=mb.ActivationFunctionType.Relu,
                ).then_inc(act_s, 1)

        @block.sync
        def _(eng):
            for B in range(NB):
                eng.wait_ge(dvo_s, B + 1)
                eng.dma_start(
                    out=out_d[B * 128 : (B + 1) * 128, :], in_=otile[B % 2][:]
                ).then_inc(out_s, 16)
            eng.wait_ge(out_s, 16 * NB)

    nc.compile()
    return nc


def _host_inputs(h, W, b, idx_arrs, dv_arrs, deg_arrs):
    cst = np.zeros((128, 257), np.float32)
    cst[:, 0:128] = np.arange(128, dtype=np.float32)[None, :]
    cst[:, 128:256] = np.eye(128, dtype=np.float32)
    cst[:, 256] = 1.0
    wa = np.concatenate([W.T.astype(np.float32), b.astype(np.float32)[None, :]], axis=0)
    in_maps = []
    for c in range(NCORES):
        in_maps.append({
            "h": np.ascontiguousarray(h.astype(np.float32)),
            "idx": idx_arrs[c],
            "dv": dv_arrs[c],
            "cst": cst,
            "wa": wa,
            "rdeg": deg_arrs[c][0],
            "ind": deg_arrs[c][1],
        })
    return in_maps


def kernel(h, edge_src, edge_dst, W, b):
    h = np.asarray(h, np.float32)
    edge_src = np.asarray(edge_src, np.int32)
    edge_dst = np.asarray(edge_dst, np.int32)
    W = np.asarray(W, np.float32)
    b = np.asarray(b, np.float32)

    from concourse.bass_utils import run_bass_kernel_spmd

    plan, instr_chunks, tot_chunks, idx_w, idx_arrs, dv_arrs, deg_arrs = _host_partition(
        edge_src, edge_dst
    )
    nc = _build_nc(plan, instr_chunks, tot_chunks, idx_w)
    in_maps = _host_inputs(h, W, b, idx_arrs, dv_arrs, deg_arrs)
    res = run_bass_kernel_spmd(nc, in_maps, list(range(NCORES)))
    out = np.concatenate(
        [res.results[c]["out"][:NPC] for c in range(NCORES)], axis=0
    )
    return out.astype(np.float32)



# revision 13
# speedup vs baseline: 1.2517x; 1.2447x over previous
"""GCN layer (gather -> mean-aggregate -> linear -> relu) on 8 TRN2 NeuronCores.

Strategy (v2):
- Nodes/outputs sharded by destination across 8 cores (12500 dsts each, 98
  blocks of 128); edges partitioned by destination core. h and the 64x64
  weight are replicated.
- out = relu(diag(rdeg) * (mean_agg(h) @ W.T) + ind*b): the linear layer is
  applied after aggregation per 128-dst block; rdeg/ind (1/deg, deg>0) are
  host-precomputed.
- h is stored in HBM as bf16 [100000, 128] rows (features duplicated to honor
  the 256B dma_gather element minimum). Edges are gathered per (dst-block,
  src-group) cell with src-groups of 25000 so indices fit int16.
- Big gather instructions: cells are packed into ~2900-index dma_gather
  instructions (descriptor ring enlarged via dynamic_dma_scratch_size) to
  amortize the ~2us fixed SWDGE latency; 4 queues are rotated per run.
- Aggregation: one-hot selection matmuls in bf16 (sel built by DVE in a
  2x-perf-eligible chunk-minor layout), accumulated per dst-block in PSUM.
- Output written transposed [64, 12544] so the final per-block transpose is
  skipped; host transposes for free.
"""

import numpy as np
from contextlib import ExitStack

N_NODES = 100000
N_EDGES = 1600000
D = 64
NCORES = 8
NPC = N_NODES // NCORES          # dsts per core (12500)
NB = (NPC + 127) // 128          # dst blocks per core (98)
GS = 25000                       # src group size (int16 index range)
NG = N_NODES // GS               # src groups (4)
CAPMAX = 1024                    # max idxs per gather instruction (= ring)
SCRATCH = 16384                  # dynamic_dma_scratch_size -> 3072 descs/queue
ARENA_DEPTH = 3                  # gather arena ring (runs)
SEL_DEPTH = 2                    # sel ring (runs)
ARENA_CH_MAX = 88                # max chunks per run (SBUF budget)
NC_RUN_MAX = 92                  # max sel cols per run (SBUF budget)


def _round16(x):
    return (np.asarray(x) + 15) & ~15


class _Plan:
    pass


def _host_plan(edge_src, edge_dst):
    core = edge_dst // NPC
    percore = []
    counts = np.zeros((NCORES, NB, NG), np.int64)
    for c in range(NCORES):
        m = np.nonzero(core == c)[0]
        src_c = edge_src[m].astype(np.int64)
        dstin = (edge_dst[m] - c * NPC).astype(np.int64)
        B = dstin >> 7
        g = src_c // GS
        order = np.lexsort((g, B))
        src_c, dstin, B, g = src_c[order], dstin[order], B[order], g[order]
        cell = B * NG + g
        counts[c] = np.bincount(cell, minlength=NB * NG).reshape(NB, NG)
        # start offset of each cell in the sorted per-core arrays
        cstarts = np.zeros(NB * NG + 1, np.int64)
        np.cumsum(counts[c].reshape(-1), out=cstarts[1:])
        percore.append((src_c, dstin, cstarts))
    caps = _round16(counts.max(axis=0)).astype(np.int64)  # [NB, NG]

    # pack blocks into runs subject to per-instr and per-run budgets
    runs = []
    b = 0
    while b < NB:
        gsum = caps[b].copy()
        b1 = b + 1
        while b1 < NB:
            t = gsum + caps[b1]
            kb = np.sum((t + 127) // 128)
            # cols estimate: chunks + one straddle per cell
            ncol = kb + (b1 + 1 - b) * NG
            if np.all(t <= CAPMAX) and kb <= ARENA_CH_MAX and ncol <= NC_RUN_MAX:
                gsum = t
                b1 += 1
            else:
                break
        runs.append((b, b1))
        b = b1

    p = _Plan()
    p.runs = runs
    p.instrs = []        # dicts: r,g,n,kb,ch0 (chunk off in run slot),q,wofs,c0,ncols
    p.block_mm = [[] for _ in range(NB)]   # per B: (r, cr, ch_in_slot)
    p.cell_slot = {}     # (r,g,B) -> slot offset of cell within instr
    colmaps = []         # per instr: dict (k, B) -> cr (run-local col idx)
    wofs = 0
    c0 = 0
    arena_ch = 0
    ncrun_max = 0
    nci_max = 0
    for r, (b0, b1) in enumerate(runs):
        ch0 = 0
        cr_base = 0
        for g in range(NG):
            cc = caps[b0:b1, g]
            n = int(cc.sum())
            kb = (n + 127) // 128
            starts = np.zeros(b1 - b0 + 1, np.int64)
            np.cumsum(cc, out=starts[1:])
            colmap = {}
            ncols = 0
            # cols: for each chunk, each overlapping cell (ordered by B)
            for k in range(kb):
                lo, hi = k * 128, (k + 1) * 128
                i0 = int(np.searchsorted(starts, lo, "right") - 1)
                i1 = int(np.searchsorted(starts, hi, "left"))
                for bi in range(i0, min(i1, b1 - b0)):
                    colmap[(k, b0 + bi)] = ncols  # instr-local col index
                    p.block_mm[b0 + bi].append((r, cr_base + ncols, ch0 + k))
                    ncols += 1
            for bi in range(b1 - b0):
                p.cell_slot[(r, g, b0 + bi)] = int(starts[bi])
            p.instrs.append(dict(r=r, g=g, n=n, kb=kb, ch0=ch0,
                                 q=(r + g) % 4, wofs=wofs, c0=c0, ncols=ncols))
            colmaps.append(colmap)
            wofs += n // 16
            c0 += ncols
            ch0 += kb
            cr_base += ncols
            nci_max = max(nci_max, ncols)
        arena_ch = max(arena_ch, ch0)
        ncrun_max = max(ncrun_max, cr_base)
    p.idx_w = wofs
    p.totcol = c0
    p.arena_ch = arena_ch
    p.ncrun = ncrun_max
    p.nci_max = nci_max
    p.caps = caps
    p.nruns = len(runs)

    # first/last flags per block
    p.block_sched = []
    for B in range(NB):
        mml = p.block_mm[B]
        p.block_sched.append(
            [(r, cr, ch, j == 0, j == len(mml) - 1) for j, (r, cr, ch) in enumerate(mml)]
        )

    # per-core data arrays
    idx_arrs, dv_arrs, deg_arrs = [], [], []
    for c in range(NCORES):
        src_c, dstin, cstarts = percore[c]
        idx16 = np.zeros((16, p.idx_w), np.int16)
        dv = np.full((128, p.totcol), -1.0, np.float32)
        for i, ins in enumerate(p.instrs):
            r, g = ins["r"], ins["g"]
            b0, b1 = runs[r]
            flat = np.zeros(ins["n"], np.int16)
            colmap = colmaps[i]
            for B in range(b0, b1):
                ci = B * NG + g
                cnt = int(counts[c, B, g])
                if cnt == 0:
                    continue
                e0 = int(cstarts[ci])
                s0 = p.cell_slot[(r, g, B)]
                flat[s0:s0 + cnt] = (src_c[e0:e0 + cnt] - g * GS).astype(np.int16)
                slots = s0 + np.arange(cnt)
                ks = slots >> 7
                ps = slots & 127
                gcs = np.array([colmap[(int(k), B)] for k in ks]) + ins["c0"]
                dv[ps, gcs] = (dstin[e0:e0 + cnt] - (B << 7)).astype(np.float32)
            w = ins["n"] // 16
            idx16[:, ins["wofs"]:ins["wofs"] + w] = flat.reshape(w, 16).T
        idx_arrs.append(np.tile(idx16, (8, 1)))
        dv_arrs.append(dv.astype(np.float32))

        deg = np.bincount(dstin, minlength=NB * 128).astype(np.float32)
        rdeg = 1.0 / np.maximum(deg, 1.0)
        ind = np.minimum(deg, 1.0)
        deg_arrs.append((rdeg.reshape(NB, 128).T.copy(), ind.reshape(NB, 128).T.copy()))
    return p, idx_arrs, dv_arrs, deg_arrs


def _build_nc(p):
    import concourse.bacc as bacc
    import concourse.mybir as mybir
    from concourse._compat import get_trn_type

    f32 = mybir.dt.float32
    bf16 = mybir.dt.bfloat16
    i16 = mybir.dt.int16

    nc = bacc.Bacc(get_trn_type() or "TRN2", debug=True, num_swdge_queues=4,
                   dynamic_dma_scratch_size=SCRATCH)
    h_d = nc.declare_dram_parameter("h", [N_NODES, 128], bf16, isOutput=False)
    idx_d = nc.declare_dram_parameter("idx", [128, p.idx_w], i16, isOutput=False)
    dv_d = nc.declare_dram_parameter("dv", [128, p.totcol], bf16, isOutput=False)
    iota_d = nc.declare_dram_parameter("iota", [128, 128 * p.nci_max], bf16, isOutput=False)
    cst_d = nc.declare_dram_parameter("cst", [128, 128], f32, isOutput=False)
    wa_d = nc.declare_dram_parameter("wa", [65, D], bf16, isOutput=False)
    rdeg_d = nc.declare_dram_parameter("rdeg", [128, NB], f32, isOutput=False)
    ind_d = nc.declare_dram_parameter("ind", [128, NB], f32, isOutput=False)
    out_d = nc.declare_dram_parameter("out", [D, NB * 128], f32, isOutput=True)

    NCR = p.ncrun
    ACH = p.arena_ch

    with ExitStack() as st:
        e = st.enter_context
        idx_sb = e(nc.sbuf_tensor("idx_sb", [128, p.idx_w], i16))
        dv_sb = e(nc.sbuf_tensor("dv_sb", [128, p.totcol], bf16))
        iota_sb = e(nc.sbuf_tensor("iota_sb", [128, 128 * p.nci_max], bf16))
        cst_sb = e(nc.sbuf_tensor("cst_sb", [128, 128], f32))
        wa_sb = e(nc.sbuf_tensor("wa_sb", [65, D], bf16))
        rdeg_sb = e(nc.sbuf_tensor("rdeg_sb", [128, NB], f32))
        ind_sb = e(nc.sbuf_tensor("ind_sb", [128, NB], f32))
        arena = e(nc.sbuf_tensor("arena", [128, ARENA_DEPTH * ACH * 128], bf16))
        sel_sb = e(nc.sbuf_tensor("sel_sb", [128, SEL_DEPTH * NCR * 128], bf16))
        agg = [e(nc.sbuf_tensor(f"agg{i}", [128, 65], f32)) for i in range(2)]
        aggT = [e(nc.sbuf_tensor(f"aggT{i}", [65, 128], bf16)) for i in range(2)]
        ysb = [e(nc.sbuf_tensor(f"ysb{i}", [64, 128], f32)) for i in range(2)]

        acc = [e(nc.psum_tensor(f"acc{i}", [128, 64], f32)) for i in range(2)]
        pt1 = [e(nc.psum_tensor(f"pt1_{i}", [65, 128], f32)) for i in range(2)]
        pmw = [e(nc.psum_tensor(f"pmw{i}", [64, 128], f32)) for i in range(2)]

        in_s = e(nc.semaphore("in_s"))
        g_s = [e(nc.semaphore(f"g_s{i}")) for i in range(4)]
        sel_s = e(nc.semaphore("sel_s"))
        blk_s = e(nc.semaphore("blk_s"))
        dep_s = e(nc.semaphore("dep_s"))
        pt1_s = e(nc.semaphore("pt1_s"))
        dt1_s = e(nc.semaphore("dt1_s"))
        pmw_s = e(nc.semaphore("pmw_s"))
        act_s = e(nc.semaphore("act_s"))
        out_p = [e(nc.semaphore(f"out_p{i}")) for i in range(2)]
        ms_s = e(nc.semaphore("ms_s"))
        block = e(nc.Block())

        def sel_view(r):
            base = (r % SEL_DEPTH) * NCR * 128
            return sel_sb[:, base:base + NCR * 128].rearrange(
                "p (d c) -> p d c", c=NCR)

        def arena_chunk(r, ch):
            base = ((r % ARENA_DEPTH) * ACH + ch) * 128
            return arena[:, base:base + 64]

        # run index of each block
        run_of = np.zeros(NB, np.int64)
        for r, (b0, b1) in enumerate(p.runs):
            run_of[b0:b1] = r

        @block.gpsimd
        def _(eng):
            eng.wait_ge(in_s, 112)  # all inputs loaded
            eng.wait_ge(ms_s, 1)    # arena zeroed
            for i, ins in enumerate(p.instrs):
                r = ins["r"]
                if ins["g"] == 0 and r >= ARENA_DEPTH:
                    eng.wait_ge(blk_s, p.runs[r - ARENA_DEPTH][1])
                if r >= 1:
                    eng.wait_ge(g_s[ins["q"]], 16 * r)
                base = ((r % ARENA_DEPTH) * ACH + ins["ch0"]) * 128
                eng.dma_gather(
                    out_ap=arena[:, base:base + ins["kb"] * 128].rearrange(
                        "p (k d) -> p k d", d=128),
                    in_ap=h_d[ins["g"] * GS:(ins["g"] + 1) * GS, :],
                    idxs_ap=idx_sb[:, ins["wofs"]:ins["wofs"] + ins["n"] // 16],
                    num_idxs=ins["n"],
                    num_idxs_reg=ins["n"],
                    elem_size=128,
                    queue_num=ins["q"],
                ).then_inc(g_s[ins["q"]], 16)

        @block.vector
        def _(eng):
            import concourse.mybir as mb
            eng.memset(arena[:], 0.0).then_inc(ms_s, 1)
            eng.wait_ge(in_s, 112)

            def tail(B):
                pb = B % 2
                eng.wait_ge(blk_s, B + 1)
                eng.tensor_copy(out=agg[pb][:, 64:65], in_=ind_sb[:, B:B + 1])
                eng.tensor_scalar(
                    out=agg[pb][:, 0:64], in0=acc[pb][:, 0:64],
                    scalar1=rdeg_sb[:, B:B + 1], scalar2=None,
                    op0=mb.AluOpType.mult,
                ).then_inc(dep_s, 1)
                eng.wait_ge(pt1_s, B + 1)
                eng.tensor_copy(out=aggT[pb][:], in_=pt1[pb][:]).then_inc(dt1_s, 1)

            ii = 0
            for r, (b0, b1) in enumerate(p.runs):
                if r >= SEL_DEPTH:
                    eng.wait_ge(blk_s, p.runs[r - SEL_DEPTH][1])
                sv = sel_view(r)
                cofs = 0
                for g in range(NG):
                    ins = p.instrs[ii]
                    ncols = ins["ncols"]
                    in0 = dv_sb[:, ins["c0"]:ins["c0"] + ncols].unsqueeze(1) \
                        .to_broadcast([128, 128, ncols])
                    in1 = iota_sb[:, :].rearrange(
                        "p (d c) -> p d c", c=p.nci_max)[:, :, 0:ncols]
                    eng.tensor_tensor(
                        out=sv[:, :, cofs:cofs + ncols],
                        in0=in0, in1=in1,
                        op=mb.AluOpType.is_equal,
                    ).then_inc(sel_s, 1)
                    cofs += ncols
                    ii += 1
                if r >= 1:
                    for B in range(*p.runs[r - 1]):
                        tail(B)
            for B in range(*p.runs[-1]):
                tail(B)

        @block.tensor
        def _(eng):
            eng.wait_ge(in_s, 112)

            def pe_ep(B):
                pb = B % 2
                eng.wait_ge(dep_s, B + 1)
                if B >= 2:
                    eng.wait_ge(dt1_s, B - 1)
                eng.matmul(
                    out=pt1[pb][:], lhsT=agg[pb][:], rhs=cst_sb[:, :],
                    is_transpose=True,
                ).then_inc(pt1_s, 1)
                eng.wait_ge(dt1_s, B + 1)
                if B >= 2:
                    eng.wait_ge(act_s, B - 1)
                eng.matmul(
                    out=pmw[pb][:], lhsT=wa_sb[:], rhs=aggT[pb][:],
                    start=True, stop=True,
                ).then_inc(pmw_s, 1)

            for r, (b0, b1) in enumerate(p.runs):
                for q in range(4):
                    eng.wait_ge(g_s[q], 16 * (r + 1))
                eng.wait_ge(sel_s, NG * (r + 1))
                sv = sel_view(r)
                for B in range(b0, b1):
                    for (rr, cr, ch, first, last) in p.block_sched[B]:
                        assert rr == r
                        if first and B >= 2:
                            eng.wait_ge(dep_s, B - 1)
                        mm = eng.matmul(
                            out=acc[B % 2][:, 0:64],
                            lhsT=sv[:, :, cr],
                            rhs=arena_chunk(r, ch),
                            start=first, stop=last,
                        )
                        if last:
                            mm.then_inc(blk_s, 1)
                    if B >= 1:
                        pe_ep(B - 1)
            pe_ep(NB - 1)

        @block.scalar
        def _(eng):
            import concourse.mybir as mb
            for B in range(NB):
                eng.wait_ge(pmw_s, B + 1)
                if B >= 2:
                    eng.wait_ge(out_p[B % 2], 16 * (B // 2))
                eng.activation(
                    out=ysb[B % 2][:], in_=pmw[B % 2][:],
                    func=mb.ActivationFunctionType.Relu,
                ).then_inc(act_s, 1)

        @block.sync
        def _(eng):
            eng.dma_start(out=idx_sb[:], in_=idx_d[:]).then_inc(in_s, 16)
            eng.dma_start(out=dv_sb[:], in_=dv_d[:]).then_inc(in_s, 16)
            eng.dma_start(out=iota_sb[:], in_=iota_d[:]).then_inc(in_s, 16)
            eng.dma_start(out=cst_sb[:], in_=cst_d[:]).then_inc(in_s, 16)
            eng.dma_start(out=wa_sb[:], in_=wa_d[:]).then_inc(in_s, 16)
            eng.dma_start(out=rdeg_sb[:], in_=rdeg_d[:]).then_inc(in_s, 16)
            eng.dma_start(out=ind_sb[:], in_=ind_d[:]).then_inc(in_s, 16)
            for B in range(NB):
                eng.wait_ge(act_s, B + 1)
                eng.dma_start(
                    out=out_d[:, B * 128:(B + 1) * 128], in_=ysb[B % 2][:]
                ).then_inc(out_p[B % 2], 16)
            eng.wait_ge(out_p[0], 16 * ((NB + 1) // 2))
            eng.wait_ge(out_p[1], 16 * (NB // 2))

    nc.compile()
    return nc


def _host_inputs(h, W, b, p, idx_arrs, dv_arrs, deg_arrs):
    import ml_dtypes
    bf = ml_dtypes.bfloat16
    h_dual = np.zeros((N_NODES, 128), bf)
    hb = h.astype(bf)
    h_dual[:, 0:64] = hb
    h_dual[:, 64:128] = hb
    iota = np.zeros((128, 128 * p.nci_max), bf)
    iota[:] = np.repeat(np.arange(128, dtype=np.float32), p.nci_max)[None, :].astype(bf)
    cst = np.eye(128, dtype=np.float32)
    wa = np.concatenate([W.T.astype(np.float32), b.astype(np.float32)[None, :]],
                        axis=0).astype(bf)
    in_maps = []
    for c in range(NCORES):
        in_maps.append({
            "h": h_dual,
            "idx": idx_arrs[c],
            "dv": dv_arrs[c].astype(bf),
            "iota": iota,
            "cst": cst,
            "wa": wa,
            "rdeg": deg_arrs[c][0],
            "ind": deg_arrs[c][1],
        })
    return in_maps


def kernel(h, edge_src, edge_dst, W, b):
    h = np.asarray(h, np.float32)
    edge_src = np.asarray(edge_src, np.int32)
    edge_dst = np.asarray(edge_dst, np.int32)
    W = np.asarray(W, np.float32)
    b = np.asarray(b, np.float32)

    from concourse.bass_utils import run_bass_kernel_spmd

    p, idx_arrs, dv_arrs, deg_arrs = _host_plan(edge_src, edge_dst)
    nc = _build_nc(p)
    in_maps = _host_inputs(h, W, b, p, idx_arrs, dv_arrs, deg_arrs)
    res = run_bass_kernel_spmd(nc, in_maps, list(range(NCORES)))
    out = np.concatenate(
        [np.asarray(res.results[c]["out"])[:, :NPC].T for c in range(NCORES)], axis=0
    )
    return np.ascontiguousarray(out.astype(np.float32))


# revision 14
# speedup vs baseline: 1.2994x; 1.0381x over previous
"""GCN layer (gather -> mean-aggregate -> linear -> relu) on 8 TRN2 NeuronCores.

Strategy (v2):
- Nodes/outputs sharded by destination across 8 cores (12500 dsts each, 98
  blocks of 128); edges partitioned by destination core. h and the 64x64
  weight are replicated.
- out = relu(diag(rdeg) * (mean_agg(h) @ W.T) + ind*b): the linear layer is
  applied after aggregation per 128-dst block; rdeg/ind (1/deg, deg>0) are
  host-precomputed.
- h is stored in HBM as bf16 [100000, 128] rows (features duplicated to honor
  the 256B dma_gather element minimum). Edges are gathered per (dst-block,
  src-group) cell with src-groups of 25000 so indices fit int16.
- Big gather instructions: cells are packed into ~2900-index dma_gather
  instructions (descriptor ring enlarged via dynamic_dma_scratch_size) to
  amortize the ~2us fixed SWDGE latency; 4 queues are rotated per run.
- Aggregation: one-hot selection matmuls in bf16 (sel built by DVE in a
  2x-perf-eligible chunk-minor layout), accumulated per dst-block in PSUM.
- Output written transposed [64, 12544] so the final per-block transpose is
  skipped; host transposes for free.
"""

import numpy as np
from contextlib import ExitStack

N_NODES = 100000
N_EDGES = 1600000
D = 64
NCORES = 8
NPC = N_NODES // NCORES          # dsts per core (12500)
NB = (NPC + 127) // 128          # dst blocks per core (98)
GS = 25000                       # src group size (int16 index range)
NG = N_NODES // GS               # src groups (4)
CAPMAX = 1024                    # max idxs per gather instruction (= ring)
SCRATCH = 49152                  # dynamic_dma_scratch_size -> 3072 descs/queue
ARENA_DEPTH = 3                  # gather arena ring (runs)
SEL_DEPTH = 2                    # sel ring (runs)
ARENA_CH_MAX = 88                # max chunks per run (SBUF budget)
NC_RUN_MAX = 92                  # max sel cols per run (SBUF budget)


def _round16(x):
    return (np.asarray(x) + 15) & ~15


class _Plan:
    pass


def _host_plan(edge_src, edge_dst):
    core = edge_dst // NPC
    percore = []
    counts = np.zeros((NCORES, NB, NG), np.int64)
    for c in range(NCORES):
        m = np.nonzero(core == c)[0]
        src_c = edge_src[m].astype(np.int64)
        dstin = (edge_dst[m] - c * NPC).astype(np.int64)
        B = dstin >> 7
        g = src_c // GS
        order = np.lexsort((g, B))
        src_c, dstin, B, g = src_c[order], dstin[order], B[order], g[order]
        cell = B * NG + g
        counts[c] = np.bincount(cell, minlength=NB * NG).reshape(NB, NG)
        # start offset of each cell in the sorted per-core arrays
        cstarts = np.zeros(NB * NG + 1, np.int64)
        np.cumsum(counts[c].reshape(-1), out=cstarts[1:])
        percore.append((src_c, dstin, cstarts))
    caps = _round16(counts.max(axis=0)).astype(np.int64)  # [NB, NG]

    # pack blocks into runs subject to per-instr and per-run budgets
    runs = []
    b = 0
    while b < NB:
        gsum = caps[b].copy()
        b1 = b + 1
        while b1 < NB:
            t = gsum + caps[b1]
            kb = np.sum((t + 127) // 128)
            # cols estimate: chunks + one straddle per cell
            ncol = kb + (b1 + 1 - b) * NG
            if np.all(t <= CAPMAX) and kb <= ARENA_CH_MAX and ncol <= NC_RUN_MAX:
                gsum = t
                b1 += 1
            else:
                break
        runs.append((b, b1))
        b = b1

    p = _Plan()
    p.runs = runs
    p.instrs = []        # dicts: r,g,n,kb,ch0 (chunk off in run slot),q,wofs,c0,ncols
    p.block_mm = [[] for _ in range(NB)]   # per B: (r, cr, ch_in_slot)
    p.cell_slot = {}     # (r,g,B) -> slot offset of cell within instr
    colmaps = []         # per instr: dict (k, B) -> cr (run-local col idx)
    wofs = 0
    c0 = 0
    arena_ch = 0
    ncrun_max = 0
    nci_max = 0
    for r, (b0, b1) in enumerate(runs):
        ch0 = 0
        cr_base = 0
        for g in range(NG):
            cc = caps[b0:b1, g]
            n = int(cc.sum())
            kb = (n + 127) // 128
            starts = np.zeros(b1 - b0 + 1, np.int64)
            np.cumsum(cc, out=starts[1:])
            colmap = {}
            ncols = 0
            # cols: for each chunk, each overlapping cell (ordered by B)
            for k in range(kb):
                lo, hi = k * 128, (k + 1) * 128
                i0 = int(np.searchsorted(starts, lo, "right") - 1)
                i1 = int(np.searchsorted(starts, hi, "left"))
                for bi in range(i0, min(i1, b1 - b0)):
                    colmap[(k, b0 + bi)] = ncols  # instr-local col index
                    p.block_mm[b0 + bi].append((r, cr_base + ncols, ch0 + k))
                    ncols += 1
            for bi in range(b1 - b0):
                p.cell_slot[(r, g, b0 + bi)] = int(starts[bi])
            p.instrs.append(dict(r=r, g=g, n=n, kb=kb, ch0=ch0,
                                 q=(r + g) % 4, wofs=wofs, c0=c0, ncols=ncols))
            colmaps.append(colmap)
            wofs += n // 16
            c0 += ncols
            ch0 += kb
            cr_base += ncols
            nci_max = max(nci_max, ncols)
        arena_ch = max(arena_ch, ch0)
        ncrun_max = max(ncrun_max, cr_base)
    p.idx_w = wofs
    p.totcol = c0
    p.arena_ch = arena_ch
    p.ncrun = ncrun_max
    p.nci_max = nci_max
    p.caps = caps
    p.nruns = len(runs)

    # first/last flags per block
    p.block_sched = []
    for B in range(NB):
        mml = p.block_mm[B]
        p.block_sched.append(
            [(r, cr, ch, j == 0, j == len(mml) - 1) for j, (r, cr, ch) in enumerate(mml)]
        )

    # per-core data arrays
    idx_arrs, dv_arrs, deg_arrs = [], [], []
    for c in range(NCORES):
        src_c, dstin, cstarts = percore[c]
        idx16 = np.zeros((16, p.idx_w), np.int16)
        dv = np.full((128, p.totcol), -1.0, np.float32)
        for i, ins in enumerate(p.instrs):
            r, g = ins["r"], ins["g"]
            b0, b1 = runs[r]
            flat = np.zeros(ins["n"], np.int16)
            colmap = colmaps[i]
            for B in range(b0, b1):
                ci = B * NG + g
                cnt = int(counts[c, B, g])
                if cnt == 0:
                    continue
                e0 = int(cstarts[ci])
                s0 = p.cell_slot[(r, g, B)]
                flat[s0:s0 + cnt] = (src_c[e0:e0 + cnt] - g * GS).astype(np.int16)
                slots = s0 + np.arange(cnt)
                ks = slots >> 7
                ps = slots & 127
                gcs = np.array([colmap[(int(k), B)] for k in ks]) + ins["c0"]
                dv[ps, gcs] = (dstin[e0:e0 + cnt] - (B << 7)).astype(np.float32)
            w = ins["n"] // 16
            idx16[:, ins["wofs"]:ins["wofs"] + w] = flat.reshape(w, 16).T
        idx_arrs.append(np.tile(idx16, (8, 1)))
        dv_arrs.append(dv.astype(np.float32))

        deg = np.bincount(dstin, minlength=NB * 128).astype(np.float32)
        rdeg = 1.0 / np.maximum(deg, 1.0)
        ind = np.minimum(deg, 1.0)
        deg_arrs.append((rdeg.reshape(NB, 128).T.copy(), ind.reshape(NB, 128).T.copy()))
    return p, idx_arrs, dv_arrs, deg_arrs


def _build_nc(p):
    import concourse.bacc as bacc
    import concourse.mybir as mybir
    from concourse._compat import get_trn_type

    f32 = mybir.dt.float32
    bf16 = mybir.dt.bfloat16
    i16 = mybir.dt.int16

    nc = bacc.Bacc(get_trn_type() or "TRN2", debug=True, num_swdge_queues=4,
                   dynamic_dma_scratch_size=SCRATCH)
    h_d = nc.declare_dram_parameter("h", [N_NODES, 128], bf16, isOutput=False)
    idx_d = nc.declare_dram_parameter("idx", [128, p.idx_w], i16, isOutput=False)
    dv_d = nc.declare_dram_parameter("dv", [128, p.totcol], bf16, isOutput=False)
    iota_d = nc.declare_dram_parameter("iota", [128, 128 * p.nci_max], bf16, isOutput=False)
    cst_d = nc.declare_dram_parameter("cst", [128, 128], f32, isOutput=False)
    wa_d = nc.declare_dram_parameter("wa", [65, D], bf16, isOutput=False)
    rdeg_d = nc.declare_dram_parameter("rdeg", [128, NB], f32, isOutput=False)
    ind_d = nc.declare_dram_parameter("ind", [128, NB], f32, isOutput=False)
    out_d = nc.declare_dram_parameter("out", [D, NB * 128], f32, isOutput=True)

    NCR = p.ncrun
    ACH = p.arena_ch

    with ExitStack() as st:
        e = st.enter_context
        idx_sb = e(nc.sbuf_tensor("idx_sb", [128, p.idx_w], i16))
        dv_sb = e(nc.sbuf_tensor("dv_sb", [128, p.totcol], bf16))
        iota_sb = e(nc.sbuf_tensor("iota_sb", [128, 128 * p.nci_max], bf16))
        cst_sb = e(nc.sbuf_tensor("cst_sb", [128, 128], f32))
        wa_sb = e(nc.sbuf_tensor("wa_sb", [65, D], bf16))
        rdeg_sb = e(nc.sbuf_tensor("rdeg_sb", [128, NB], f32))
        ind_sb = e(nc.sbuf_tensor("ind_sb", [128, NB], f32))
        arena = e(nc.sbuf_tensor("arena", [128, ARENA_DEPTH * ACH * 128], bf16))
        sel_sb = e(nc.sbuf_tensor("sel_sb", [128, SEL_DEPTH * NCR * 128], bf16))
        agg = [e(nc.sbuf_tensor(f"agg{i}", [128, 65], f32)) for i in range(2)]
        aggT = [e(nc.sbuf_tensor(f"aggT{i}", [65, 128], bf16)) for i in range(2)]
        ysb = [e(nc.sbuf_tensor(f"ysb{i}", [64, 128], f32)) for i in range(2)]

        acc = [e(nc.psum_tensor(f"acc{i}", [128, 64], f32)) for i in range(2)]
        pt1 = [e(nc.psum_tensor(f"pt1_{i}", [65, 128], f32)) for i in range(2)]
        pmw = [e(nc.psum_tensor(f"pmw{i}", [64, 128], f32)) for i in range(2)]

        in_s = e(nc.semaphore("in_s"))
        g_s = [e(nc.semaphore(f"g_s{i}")) for i in range(4)]
        sel_s = e(nc.semaphore("sel_s"))
        blk_s = e(nc.semaphore("blk_s"))
        dep_s = e(nc.semaphore("dep_s"))
        pt1_s = e(nc.semaphore("pt1_s"))
        dt1_s = e(nc.semaphore("dt1_s"))
        pmw_s = e(nc.semaphore("pmw_s"))
        act_s = e(nc.semaphore("act_s"))
        out_p = [e(nc.semaphore(f"out_p{i}")) for i in range(2)]
        ms_s = e(nc.semaphore("ms_s"))
        block = e(nc.Block())

        def sel_view(r):
            base = (r % SEL_DEPTH) * NCR * 128
            return sel_sb[:, base:base + NCR * 128].rearrange(
                "p (d c) -> p d c", c=NCR)

        def arena_chunk(r, ch):
            base = ((r % ARENA_DEPTH) * ACH + ch) * 128
            return arena[:, base:base + 64]

        # run index of each block
        run_of = np.zeros(NB, np.int64)
        for r, (b0, b1) in enumerate(p.runs):
            run_of[b0:b1] = r

        @block.gpsimd
        def _(eng):
            eng.wait_ge(in_s, 112)  # all inputs loaded
            eng.wait_ge(ms_s, 1)    # arena zeroed
            for i, ins in enumerate(p.instrs):
                r = ins["r"]
                if ins["g"] == 0 and r >= ARENA_DEPTH:
                    eng.wait_ge(blk_s, p.runs[r - ARENA_DEPTH][1])
                if r >= 1:
                    eng.wait_ge(g_s[ins["q"]], 16 * r)
                base = ((r % ARENA_DEPTH) * ACH + ins["ch0"]) * 128
                eng.dma_gather(
                    out_ap=arena[:, base:base + ins["kb"] * 128].rearrange(
                        "p (k d) -> p k d", d=128),
                    in_ap=h_d[ins["g"] * GS:(ins["g"] + 1) * GS, :],
                    idxs_ap=idx_sb[:, ins["wofs"]:ins["wofs"] + ins["n"] // 16],
                    num_idxs=ins["n"],
                    num_idxs_reg=ins["n"],
                    elem_size=128,
                    queue_num=ins["q"],
                ).then_inc(g_s[ins["q"]], 16)

        @block.vector
        def _(eng):
            import concourse.mybir as mb
            eng.memset(arena[:], 0.0).then_inc(ms_s, 1)
            eng.wait_ge(in_s, 112)

            def tail(B):
                pb = B % 2
                eng.wait_ge(blk_s, B + 1)
                eng.tensor_copy(out=agg[pb][:, 64:65], in_=ind_sb[:, B:B + 1])
                eng.tensor_scalar(
                    out=agg[pb][:, 0:64], in0=acc[pb][:, 0:64],
                    scalar1=rdeg_sb[:, B:B + 1], scalar2=None,
                    op0=mb.AluOpType.mult,
                ).then_inc(dep_s, 1)
                eng.wait_ge(pt1_s, B + 1)
                eng.tensor_copy(out=aggT[pb][:], in_=pt1[pb][:]).then_inc(dt1_s, 1)

            ii = 0
            for r, (b0, b1) in enumerate(p.runs):
                if r >= SEL_DEPTH:
                    eng.wait_ge(blk_s, p.runs[r - SEL_DEPTH][1])
                sv = sel_view(r)
                cofs = 0
                for g in range(NG):
                    ins = p.instrs[ii]
                    ncols = ins["ncols"]
                    in0 = dv_sb[:, ins["c0"]:ins["c0"] + ncols].unsqueeze(1) \
                        .to_broadcast([128, 128, ncols])
                    in1 = iota_sb[:, :].rearrange(
                        "p (d c) -> p d c", c=p.nci_max)[:, :, 0:ncols]
                    eng.tensor_tensor(
                        out=sv[:, :, cofs:cofs + ncols],
                        in0=in0, in1=in1,
                        op=mb.AluOpType.is_equal,
                    ).then_inc(sel_s, 1)
                    cofs += ncols
                    ii += 1
                if r >= 1:
                    for B in range(*p.runs[r - 1]):
                        tail(B)
            for B in range(*p.runs[-1]):
                tail(B)

        @block.tensor
        def _(eng):
            eng.wait_ge(in_s, 112)

            def pe_ep(B):
                pb = B % 2
                eng.wait_ge(dep_s, B + 1)
                if B >= 2:
                    eng.wait_ge(dt1_s, B - 1)
                eng.matmul(
                    out=pt1[pb][:], lhsT=agg[pb][:], rhs=cst_sb[:, :],
                    is_transpose=True,
                ).then_inc(pt1_s, 1)
                eng.wait_ge(dt1_s, B + 1)
                if B >= 2:
                    eng.wait_ge(act_s, B - 1)
                eng.matmul(
                    out=pmw[pb][:], lhsT=wa_sb[:], rhs=aggT[pb][:],
                    start=True, stop=True,
                ).then_inc(pmw_s, 1)

            for r, (b0, b1) in enumerate(p.runs):
                for q in range(4):
                    eng.wait_ge(g_s[q], 16 * (r + 1))
                eng.wait_ge(sel_s, NG * (r + 1))
                sv = sel_view(r)
                for B in range(b0, b1):
                    for (rr, cr, ch, first, last) in p.block_sched[B]:
                        assert rr == r
                        if first and B >= 2:
                            eng.wait_ge(dep_s, B - 1)
                        mm = eng.matmul(
                            out=acc[B % 2][:, 0:64],
                            lhsT=sv[:, :, cr],
                            rhs=arena_chunk(r, ch),
                            start=first, stop=last,
                        )
                        if last:
                            mm.then_inc(blk_s, 1)
                    if B >= 1:
                        pe_ep(B - 1)
            pe_ep(NB - 1)

        @block.scalar
        def _(eng):
            import concourse.mybir as mb
            for B in range(NB):
                eng.wait_ge(pmw_s, B + 1)
                if B >= 2:
                    eng.wait_ge(out_p[B % 2], 16 * (B // 2))
                eng.activation(
                    out=ysb[B % 2][:], in_=pmw[B % 2][:],
                    func=mb.ActivationFunctionType.Relu,
                ).then_inc(act_s, 1)

        @block.sync
        def _(eng):
            eng.dma_start(out=idx_sb[:], in_=idx_d[:]).then_inc(in_s, 16)
            eng.dma_start(out=dv_sb[:], in_=dv_d[:]).then_inc(in_s, 16)
            eng.dma_start(out=iota_sb[:], in_=iota_d[:]).then_inc(in_s, 16)
            eng.dma_start(out=cst_sb[:], in_=cst_d[:]).then_inc(in_s, 16)
            eng.dma_start(out=wa_sb[:], in_=wa_d[:]).then_inc(in_s, 16)
            eng.dma_start(out=rdeg_sb[:], in_=rdeg_d[:]).then_inc(in_s, 16)
            eng.dma_start(out=ind_sb[:], in_=ind_d[:]).then_inc(in_s, 16)
            for B in range(NB):
                eng.wait_ge(act_s, B + 1)
                eng.dma_start(
                    out=out_d[:, B * 128:(B + 1) * 128], in_=ysb[B % 2][:]
                ).then_inc(out_p[B % 2], 16)
            eng.wait_ge(out_p[0], 16 * ((NB + 1) // 2))
            eng.wait_ge(out_p[1], 16 * (NB // 2))

    nc.compile()
    return nc


def _host_inputs(h, W, b, p, idx_arrs, dv_arrs, deg_arrs):
    import ml_dtypes
    bf = ml_dtypes.bfloat16
    h_dual = np.zeros((N_NODES, 128), bf)
    hb = h.astype(bf)
    h_dual[:, 0:64] = hb
    h_dual[:, 64:128] = hb
    iota = np.zeros((128, 128 * p.nci_max), bf)
    iota[:] = np.repeat(np.arange(128, dtype=np.float32), p.nci_max)[None, :].astype(bf)
    cst = np.eye(128, dtype=np.float32)
    wa = np.concatenate([W.T.astype(np.float32), b.astype(np.float32)[None, :]],
                        axis=0).astype(bf)
    in_maps = []
    for c in range(NCORES):
        in_maps.append({
            "h": h_dual,
            "idx": idx_arrs[c],
            "dv": dv_arrs[c].astype(bf),
            "iota": iota,
            "cst": cst,
            "wa": wa,
            "rdeg": deg_arrs[c][0],
            "ind": deg_arrs[c][1],
        })
    return in_maps


def kernel(h, edge_src, edge_dst, W, b):
    h = np.asarray(h, np.float32)
    edge_src = np.asarray(edge_src, np.int32)
    edge_dst = np.asarray(edge_dst, np.int32)
    W = np.asarray(W, np.float32)
    b = np.asarray(b, np.float32)

    from concourse.bass_utils import run_bass_kernel_spmd

    p, idx_arrs, dv_arrs, deg_arrs = _host_plan(edge_src, edge_dst)
    nc = _build_nc(p)
    in_maps = _host_inputs(h, W, b, p, idx_arrs, dv_arrs, deg_arrs)
    res = run_bass_kernel_spmd(nc, in_maps, list(range(NCORES)))
    out = np.concatenate(
        [np.asarray(res.results[c]["out"])[:, :NPC].T for c in range(NCORES)], axis=0
    )
    return np.ascontiguousarray(out.astype(np.float32))


# revision 16
# speedup vs baseline: 1.4673x; 1.1292x over previous
"""GCN layer (gather -> mean-aggregate -> linear -> relu) on 8 TRN2 NeuronCores.

Strategy (v2):
- Nodes/outputs sharded by destination across 8 cores (12500 dsts each, 98
  blocks of 128); edges partitioned by destination core. h and the 64x64
  weight are replicated.
- out = relu(diag(rdeg) * (mean_agg(h) @ W.T) + ind*b): the linear layer is
  applied after aggregation per 128-dst block; rdeg/ind (1/deg, deg>0) are
  host-precomputed.
- h is stored in HBM as bf16 [100000, 128] rows (features duplicated to honor
  the 256B dma_gather element minimum). Edges are gathered per (dst-block,
  src-group) cell with src-groups of 25000 so indices fit int16.
- Big gather instructions: cells are packed into ~2900-index dma_gather
  instructions (descriptor ring enlarged via dynamic_dma_scratch_size) to
  amortize the ~2us fixed SWDGE latency; 4 queues are rotated per run.
- Aggregation: one-hot selection matmuls in bf16 (sel built by DVE in a
  2x-perf-eligible chunk-minor layout), accumulated per dst-block in PSUM.
- Output written transposed [64, 12544] so the final per-block transpose is
  skipped; host transposes for free.
"""

import numpy as np
from contextlib import ExitStack

N_NODES = 100000
N_EDGES = 1600000
D = 64
NCORES = 8
NPC = N_NODES // NCORES          # dsts per core (12500)
NB = (NPC + 127) // 128          # dst blocks per core (98)
GS = 12500                       # src group size (int16 index range)
NG = N_NODES // GS               # src groups (8)
CAPMAX = 1024                    # max idxs per gather instruction (HW ucode cap)
SCRATCH = 49152                  # dynamic_dma_scratch_size -> 3072 descs/queue
QDEPTH = 3                       # in-flight gathers per queue (ring = 3*1024)
ARENA_DEPTH = 4                  # gather arena ring (runs)
SEL_DEPTH = 2                    # sel ring (runs)
ARENA_CH_MAX = 60                # max chunks per run (SBUF budget)
NC_RUN_MAX = 84                  # max sel cols per run (SBUF budget)


def _round16(x):
    return (np.asarray(x) + 15) & ~15


class _Plan:
    pass


def _host_plan(edge_src, edge_dst):
    core = edge_dst // NPC
    percore = []
    counts = np.zeros((NCORES, NB, NG), np.int64)
    for c in range(NCORES):
        m = np.nonzero(core == c)[0]
        src_c = edge_src[m].astype(np.int64)
        dstin = (edge_dst[m] - c * NPC).astype(np.int64)
        B = dstin >> 7
        g = src_c // GS
        order = np.lexsort((g, B))
        src_c, dstin, B, g = src_c[order], dstin[order], B[order], g[order]
        cell = B * NG + g
        counts[c] = np.bincount(cell, minlength=NB * NG).reshape(NB, NG)
        # start offset of each cell in the sorted per-core arrays
        cstarts = np.zeros(NB * NG + 1, np.int64)
        np.cumsum(counts[c].reshape(-1), out=cstarts[1:])
        percore.append((src_c, dstin, cstarts))
    caps = _round16(counts.max(axis=0)).astype(np.int64)  # [NB, NG]

    # pack blocks into runs subject to per-instr and per-run budgets
    runs = []
    b = 0
    while b < NB:
        gsum = caps[b].copy()
        b1 = b + 1
        while b1 < NB:
            t = gsum + caps[b1]
            kb = np.sum((t + 127) // 128)
            # cols estimate: chunks + one straddle per cell
            ncol = kb + (b1 + 1 - b) * NG
            if np.all(t <= CAPMAX) and kb <= ARENA_CH_MAX and ncol <= NC_RUN_MAX:
                gsum = t
                b1 += 1
            else:
                break
        runs.append((b, b1))
        b = b1

    p = _Plan()
    p.runs = runs
    qseq_ctr = [0, 0, 0, 0]
    p.instrs = []        # dicts: r,g,n,kb,ch0 (chunk off in run slot),q,wofs,c0,ncols
    p.block_mm = [[] for _ in range(NB)]   # per B: (r, cr, ch_in_slot)
    p.cell_slot = {}     # (r,g,B) -> slot offset of cell within instr
    colmaps = []         # per instr: dict (k, B) -> cr (run-local col idx)
    wofs = 0
    c0 = 0
    arena_ch = 0
    ncrun_max = 0
    nci_max = 0
    for r, (b0, b1) in enumerate(runs):
        ch0 = 0
        cr_base = 0
        for g in range(NG):
            cc = caps[b0:b1, g]
            n = int(cc.sum())
            kb = (n + 127) // 128
            starts = np.zeros(b1 - b0 + 1, np.int64)
            np.cumsum(cc, out=starts[1:])
            colmap = {}
            ncols = 0
            # cols: for each chunk, each overlapping cell (ordered by B)
            for k in range(kb):
                lo, hi = k * 128, (k + 1) * 128
                i0 = int(np.searchsorted(starts, lo, "right") - 1)
                i1 = int(np.searchsorted(starts, hi, "left"))
                for bi in range(i0, min(i1, b1 - b0)):
                    colmap[(k, b0 + bi)] = ncols  # instr-local col index
                    p.block_mm[b0 + bi].append((r, cr_base + ncols, ch0 + k))
                    ncols += 1
            for bi in range(b1 - b0):
                p.cell_slot[(r, g, b0 + bi)] = int(starts[bi])
            q = (r + g) % 4
            p.instrs.append(dict(r=r, g=g, n=n, kb=kb, ch0=ch0,
                                 q=q, seq=qseq_ctr[q], wofs=wofs, c0=c0,
                                 ncols=ncols))
            qseq_ctr[q] += 1
            colmaps.append(colmap)
            wofs += n // 16
            c0 += ncols
            ch0 += kb
            cr_base += ncols
            nci_max = max(nci_max, ncols)
        arena_ch = max(arena_ch, ch0)
        ncrun_max = max(ncrun_max, cr_base)
    p.idx_w = wofs
    p.totcol = c0
    p.arena_ch = arena_ch
    p.ncrun = ncrun_max
    p.nci_max = nci_max
    p.caps = caps
    p.nruns = len(runs)

    p.run_instrs = [[] for _ in runs]
    for ins in p.instrs:
        p.run_instrs[ins["r"]].append(ins)

    # first/last flags per block
    p.block_sched = []
    for B in range(NB):
        mml = p.block_mm[B]
        p.block_sched.append(
            [(r, cr, ch, j == 0, j == len(mml) - 1) for j, (r, cr, ch) in enumerate(mml)]
        )

    # per-core data arrays
    idx_arrs, dv_arrs, deg_arrs = [], [], []
    for c in range(NCORES):
        src_c, dstin, cstarts = percore[c]
        idx16 = np.zeros((16, p.idx_w), np.int16)
        dv = np.full((128, p.totcol), -1.0, np.float32)
        for i, ins in enumerate(p.instrs):
            r, g = ins["r"], ins["g"]
            b0, b1 = runs[r]
            flat = np.zeros(ins["n"], np.int16)
            colmap = colmaps[i]
            for B in range(b0, b1):
                ci = B * NG + g
                cnt = int(counts[c, B, g])
                if cnt == 0:
                    continue
                e0 = int(cstarts[ci])
                s0 = p.cell_slot[(r, g, B)]
                flat[s0:s0 + cnt] = (src_c[e0:e0 + cnt] - g * GS).astype(np.int16)
                slots = s0 + np.arange(cnt)
                ks = slots >> 7
                ps = slots & 127
                gcs = np.array([colmap[(int(k), B)] for k in ks]) + ins["c0"]
                dv[ps, gcs] = (dstin[e0:e0 + cnt] - (B << 7)).astype(np.float32)
            w = ins["n"] // 16
            idx16[:, ins["wofs"]:ins["wofs"] + w] = flat.reshape(w, 16).T
        idx_arrs.append(np.tile(idx16, (8, 1)))
        dv_arrs.append(dv.astype(np.float32))

        deg = np.bincount(dstin, minlength=NB * 128).astype(np.float32)
        rdeg = 1.0 / np.maximum(deg, 1.0)
        ind = np.minimum(deg, 1.0)
        deg_arrs.append((rdeg.reshape(NB, 128).T.copy(), ind.reshape(NB, 128).T.copy()))
    return p, idx_arrs, dv_arrs, deg_arrs


def _build_nc(p):
    import concourse.bacc as bacc
    import concourse.mybir as mybir
    from concourse._compat import get_trn_type

    f32 = mybir.dt.float32
    bf16 = mybir.dt.bfloat16
    i16 = mybir.dt.int16

    nc = bacc.Bacc(get_trn_type() or "TRN2", debug=True, num_swdge_queues=4,
                   dynamic_dma_scratch_size=SCRATCH)
    h_d = nc.declare_dram_parameter("h", [N_NODES, 128], bf16, isOutput=False)
    idx_d = nc.declare_dram_parameter("idx", [128, p.idx_w], i16, isOutput=False)
    dv_d = nc.declare_dram_parameter("dv", [128, p.totcol], bf16, isOutput=False)
    iota_d = nc.declare_dram_parameter("iota", [128, 128 * p.nci_max], bf16, isOutput=False)
    cst_d = nc.declare_dram_parameter("cst", [128, 128], f32, isOutput=False)
    wa_d = nc.declare_dram_parameter("wa", [65, D], bf16, isOutput=False)
    rdeg_d = nc.declare_dram_parameter("rdeg", [128, NB], f32, isOutput=False)
    ind_d = nc.declare_dram_parameter("ind", [128, NB], f32, isOutput=False)
    out_d = nc.declare_dram_parameter("out", [D, NB * 128], f32, isOutput=True)

    NCR = p.ncrun
    ACH = p.arena_ch

    with ExitStack() as st:
        e = st.enter_context
        idx_sb = e(nc.sbuf_tensor("idx_sb", [128, p.idx_w], i16))
        dv_sb = e(nc.sbuf_tensor("dv_sb", [128, p.totcol], bf16))
        iota_sb = e(nc.sbuf_tensor("iota_sb", [128, 128 * p.nci_max], bf16))
        cst_sb = e(nc.sbuf_tensor("cst_sb", [128, 128], f32))
        wa_sb = e(nc.sbuf_tensor("wa_sb", [65, D], bf16))
        rdeg_sb = e(nc.sbuf_tensor("rdeg_sb", [128, NB], f32))
        ind_sb = e(nc.sbuf_tensor("ind_sb", [128, NB], f32))
        arena = e(nc.sbuf_tensor("arena", [128, ARENA_DEPTH * ACH * 128], bf16))
        sel_sb = e(nc.sbuf_tensor("sel_sb", [128, SEL_DEPTH * NCR * 128], bf16))
        agg = [e(nc.sbuf_tensor(f"agg{i}", [128, 65], f32)) for i in range(2)]
        aggT = [e(nc.sbuf_tensor(f"aggT{i}", [65, 128], bf16)) for i in range(2)]
        ysb = [e(nc.sbuf_tensor(f"ysb{i}", [64, 128], f32)) for i in range(2)]

        acc = [e(nc.psum_tensor(f"acc{i}", [128, 64], f32)) for i in range(2)]
        pt1 = [e(nc.psum_tensor(f"pt1_{i}", [65, 128], f32)) for i in range(2)]
        pmw = [e(nc.psum_tensor(f"pmw{i}", [64, 128], f32)) for i in range(2)]

        in_s = e(nc.semaphore("in_s"))
        g_q = [[e(nc.semaphore(f"g_q{q}_{d}")) for d in range(QDEPTH)]
               for q in range(4)]
        sel_s = e(nc.semaphore("sel_s"))
        blk_s = e(nc.semaphore("blk_s"))
        dep_s = e(nc.semaphore("dep_s"))
        pt1_s = e(nc.semaphore("pt1_s"))
        dt1_s = e(nc.semaphore("dt1_s"))
        pmw_s = e(nc.semaphore("pmw_s"))
        act_s = e(nc.semaphore("act_s"))
        out_p = [e(nc.semaphore(f"out_p{i}")) for i in range(2)]
        ms_s = e(nc.semaphore("ms_s"))
        block = e(nc.Block())

        def sel_view(r):
            base = (r % SEL_DEPTH) * NCR * 128
            return sel_sb[:, base:base + NCR * 128].rearrange(
                "p (d c) -> p d c", c=NCR)

        def arena_chunk(r, ch):
            base = ((r % ARENA_DEPTH) * ACH + ch) * 128
            return arena[:, base:base + 64]

        # run index of each block
        run_of = np.zeros(NB, np.int64)
        for r, (b0, b1) in enumerate(p.runs):
            run_of[b0:b1] = r

        @block.gpsimd
        def _(eng):
            eng.wait_ge(in_s, 112)  # all inputs loaded
            eng.wait_ge(ms_s, 1)    # arena zeroed
            for i, ins in enumerate(p.instrs):
                r = ins["r"]
                if ins["g"] == 0 and r >= ARENA_DEPTH:
                    eng.wait_ge(blk_s, p.runs[r - ARENA_DEPTH][1])
                sq = ins["seq"]
                if sq >= QDEPTH:
                    eng.wait_ge(g_q[ins["q"]][sq % QDEPTH], 16 * (sq // QDEPTH))
                base = ((r % ARENA_DEPTH) * ACH + ins["ch0"]) * 128
                eng.dma_gather(
                    out_ap=arena[:, base:base + ins["kb"] * 128].rearrange(
                        "p (k d) -> p k d", d=128),
                    in_ap=h_d[ins["g"] * GS:(ins["g"] + 1) * GS, :],
                    idxs_ap=idx_sb[:, ins["wofs"]:ins["wofs"] + ins["n"] // 16],
                    num_idxs=ins["n"],
                    num_idxs_reg=ins["n"],
                    elem_size=128,
                    queue_num=ins["q"],
                ).then_inc(g_q[ins["q"]][ins["seq"] % QDEPTH], 16)

        @block.vector
        def _(eng):
            import concourse.mybir as mb
            eng.memset(arena[:], 0.0).then_inc(ms_s, 1)
            eng.wait_ge(in_s, 112)

            def tail(B):
                pb = B % 2
                eng.wait_ge(blk_s, B + 1)
                eng.tensor_copy(out=agg[pb][:, 64:65], in_=ind_sb[:, B:B + 1])
                eng.tensor_scalar(
                    out=agg[pb][:, 0:64], in0=acc[pb][:, 0:64],
                    scalar1=rdeg_sb[:, B:B + 1], scalar2=None,
                    op0=mb.AluOpType.mult,
                ).then_inc(dep_s, 1)
                eng.wait_ge(pt1_s, B + 1)
                eng.tensor_copy(out=aggT[pb][:], in_=pt1[pb][:]).then_inc(dt1_s, 1)

            for r, (b0, b1) in enumerate(p.runs):
                if r >= SEL_DEPTH:
                    eng.wait_ge(blk_s, p.runs[r - SEL_DEPTH][1])
                sv = sel_view(r)
                cofs = 0
                for ins in p.run_instrs[r]:
                    ncols = ins["ncols"]
                    in0 = dv_sb[:, ins["c0"]:ins["c0"] + ncols].unsqueeze(1) \
                        .to_broadcast([128, 128, ncols])
                    in1 = iota_sb[:, :].rearrange(
                        "p (d c) -> p d c", c=p.nci_max)[:, :, 0:ncols]
                    eng.tensor_tensor(
                        out=sv[:, :, cofs:cofs + ncols],
                        in0=in0, in1=in1,
                        op=mb.AluOpType.is_equal,
                    ).then_inc(sel_s, 1)
                    cofs += ncols
                if r >= 1:
                    for B in range(*p.runs[r - 1]):
                        tail(B)
            for B in range(*p.runs[-1]):
                tail(B)

        @block.tensor
        def _(eng):
            eng.wait_ge(in_s, 112)

            def pe_ep(B):
                pb = B % 2
                eng.wait_ge(dep_s, B + 1)
                if B >= 2:
                    eng.wait_ge(dt1_s, B - 1)
                eng.matmul(
                    out=pt1[pb][:], lhsT=agg[pb][:], rhs=cst_sb[:, :],
                    is_transpose=True,
                ).then_inc(pt1_s, 1)
                eng.wait_ge(dt1_s, B + 1)
                if B >= 2:
                    eng.wait_ge(act_s, B - 1)
                eng.matmul(
                    out=pmw[pb][:], lhsT=wa_sb[:], rhs=aggT[pb][:],
                    start=True, stop=True,
                ).then_inc(pmw_s, 1)

            nsel = 0
            for r, (b0, b1) in enumerate(p.runs):
                for ins in p.run_instrs[r]:
                    sq = ins["seq"]
                    eng.wait_ge(g_q[ins["q"]][sq % QDEPTH],
                                16 * (sq // QDEPTH + 1))
                nsel += len(p.run_instrs[r])
                eng.wait_ge(sel_s, nsel)
                sv = sel_view(r)
                for B in range(b0, b1):
                    for (rr, cr, ch, first, last) in p.block_sched[B]:
                        assert rr == r
                        if first and B >= 2:
                            eng.wait_ge(dep_s, B - 1)
                        mm = eng.matmul(
                            out=acc[B % 2][:, 0:64],
                            lhsT=sv[:, :, cr],
                            rhs=arena_chunk(r, ch),
                            start=first, stop=last,
                        )
                        if last:
                            mm.then_inc(blk_s, 1)
                    if B >= 1:
                        pe_ep(B - 1)
            pe_ep(NB - 1)

        @block.scalar
        def _(eng):
            import concourse.mybir as mb
            for B in range(NB):
                eng.wait_ge(pmw_s, B + 1)
                if B >= 2:
                    eng.wait_ge(out_p[B % 2], 16 * (B // 2))
                eng.activation(
                    out=ysb[B % 2][:], in_=pmw[B % 2][:],
                    func=mb.ActivationFunctionType.Relu,
                ).then_inc(act_s, 1)

        @block.sync
        def _(eng):
            eng.dma_start(out=idx_sb[:], in_=idx_d[:]).then_inc(in_s, 16)
            eng.dma_start(out=dv_sb[:], in_=dv_d[:]).then_inc(in_s, 16)
            eng.dma_start(out=iota_sb[:], in_=iota_d[:]).then_inc(in_s, 16)
            eng.dma_start(out=cst_sb[:], in_=cst_d[:]).then_inc(in_s, 16)
            eng.dma_start(out=wa_sb[:], in_=wa_d[:]).then_inc(in_s, 16)
            eng.dma_start(out=rdeg_sb[:], in_=rdeg_d[:]).then_inc(in_s, 16)
            eng.dma_start(out=ind_sb[:], in_=ind_d[:]).then_inc(in_s, 16)
            for B in range(NB):
                eng.wait_ge(act_s, B + 1)
                eng.dma_start(
                    out=out_d[:, B * 128:(B + 1) * 128], in_=ysb[B % 2][:]
                ).then_inc(out_p[B % 2], 16)
            eng.wait_ge(out_p[0], 16 * ((NB + 1) // 2))
            eng.wait_ge(out_p[1], 16 * (NB // 2))

    nc.compile()
    return nc


def _host_inputs(h, W, b, p, idx_arrs, dv_arrs, deg_arrs):
    import ml_dtypes
    bf = ml_dtypes.bfloat16
    h_dual = np.zeros((N_NODES, 128), bf)
    hb = h.astype(bf)
    h_dual[:, 0:64] = hb
    h_dual[:, 64:128] = hb
    iota = np.zeros((128, 128 * p.nci_max), bf)
    iota[:] = np.repeat(np.arange(128, dtype=np.float32), p.nci_max)[None, :].astype(bf)
    cst = np.eye(128, dtype=np.float32)
    wa = np.concatenate([W.T.astype(np.float32), b.astype(np.float32)[None, :]],
                        axis=0).astype(bf)
    in_maps = []
    for c in range(NCORES):
        in_maps.append({
            "h": h_dual,
            "idx": idx_arrs[c],
            "dv": dv_arrs[c].astype(bf),
            "iota": iota,
            "cst": cst,
            "wa": wa,
            "rdeg": deg_arrs[c][0],
            "ind": deg_arrs[c][1],
        })
    return in_maps


def kernel(h, edge_src, edge_dst, W, b):
    h = np.asarray(h, np.float32)
    edge_src = np.asarray(edge_src, np.int32)
    edge_dst = np.asarray(edge_dst, np.int32)
    W = np.asarray(W, np.float32)
    b = np.asarray(b, np.float32)

    from concourse.bass_utils import run_bass_kernel_spmd

    p, idx_arrs, dv_arrs, deg_arrs = _host_plan(edge_src, edge_dst)
    nc = _build_nc(p)
    in_maps = _host_inputs(h, W, b, p, idx_arrs, dv_arrs, deg_arrs)
    res = run_bass_kernel_spmd(nc, in_maps, list(range(NCORES)))
    out = np.concatenate(
        [np.asarray(res.results[c]["out"])[:, :NPC].T for c in range(NCORES)], axis=0
    )
    return np.ascontiguousarray(out.astype(np.float32))


# revision 17
# speedup vs baseline: 2.1043x; 1.4341x over previous
"""GCN layer (gather -> mean-aggregate -> linear -> relu) on 8 TRN2 NeuronCores.

Strategy (v2):
- Nodes/outputs sharded by destination across 8 cores (12500 dsts each, 98
  blocks of 128); edges partitioned by destination core. h and the 64x64
  weight are replicated.
- out = relu(diag(rdeg) * (mean_agg(h) @ W.T) + ind*b): the linear layer is
  applied after aggregation per 128-dst block; rdeg/ind (1/deg, deg>0) are
  host-precomputed.
- h is stored in HBM as bf16 [100000, 128] rows (features duplicated to honor
  the 256B dma_gather element minimum). Edges are gathered per (dst-block,
  src-group) cell with src-groups of 25000 so indices fit int16.
- Big gather instructions: cells are packed into ~2900-index dma_gather
  instructions (descriptor ring enlarged via dynamic_dma_scratch_size) to
  amortize the ~2us fixed SWDGE latency; 4 queues are rotated per run.
- Aggregation: one-hot selection matmuls in bf16 (sel built by DVE in a
  2x-perf-eligible chunk-minor layout), accumulated per dst-block in PSUM.
- Output written transposed [64, 12544] so the final per-block transpose is
  skipped; host transposes for free.
"""

import numpy as np
from contextlib import ExitStack

N_NODES = 100000
N_EDGES = 1600000
D = 64
NCORES = 8
NPC = N_NODES // NCORES          # dsts per core (12500)
NB = (NPC + 127) // 128          # dst blocks per core (98)
GS = 12500                       # src group size (int16 index range)
NG = N_NODES // GS               # src groups (8)
CAPMAX = 1024                    # max idxs per gather instruction (HW ucode cap)
SCRATCH = 49152                  # dynamic_dma_scratch_size -> 3072 descs/queue
QDEPTH = 3                       # in-flight gathers per queue (ring = 3*1024)
ARENA_DEPTH = 4                  # gather arena ring (runs)
SEL_DEPTH = 2                    # sel ring (runs)
ARENA_CH_MAX = 60                # max chunks per run (SBUF budget)
NC_RUN_MAX = 84                  # max sel cols per run (SBUF budget)


def _round16(x):
    return (np.asarray(x) + 15) & ~15


class _Plan:
    pass


def _host_plan(edge_src, edge_dst):
    core = edge_dst // NPC
    percore = []
    counts = np.zeros((NCORES, NB, NG), np.int64)
    for c in range(NCORES):
        m = np.nonzero(core == c)[0]
        src_c = edge_src[m].astype(np.int64)
        dstin = (edge_dst[m] - c * NPC).astype(np.int64)
        B = dstin >> 7
        g = src_c // GS
        order = np.lexsort((src_c, g, B))  # within cell: ascending src (HBM row locality)
        src_c, dstin, B, g = src_c[order], dstin[order], B[order], g[order]
        cell = B * NG + g
        counts[c] = np.bincount(cell, minlength=NB * NG).reshape(NB, NG)
        # start offset of each cell in the sorted per-core arrays
        cstarts = np.zeros(NB * NG + 1, np.int64)
        np.cumsum(counts[c].reshape(-1), out=cstarts[1:])
        percore.append((src_c, dstin, cstarts))
    caps = _round16(counts.max(axis=0)).astype(np.int64)  # [NB, NG]

    # pack blocks into runs subject to per-instr and per-run budgets
    runs = []
    b = 0
    while b < NB:
        gsum = caps[b].copy()
        b1 = b + 1
        while b1 < NB:
            t = gsum + caps[b1]
            kb = np.sum((t + 127) // 128)
            # cols estimate: chunks + one straddle per cell
            ncol = kb + (b1 + 1 - b) * NG
            if np.all(t <= CAPMAX) and kb <= ARENA_CH_MAX and ncol <= NC_RUN_MAX:
                gsum = t
                b1 += 1
            else:
                break
        runs.append((b, b1))
        b = b1

    p = _Plan()
    p.runs = runs
    qseq_ctr = [0, 0, 0, 0]
    p.instrs = []        # dicts: r,g,n,kb,ch0 (chunk off in run slot),q,wofs,c0,ncols
    p.block_mm = [[] for _ in range(NB)]   # per B: (r, cr, ch_in_slot)
    p.cell_slot = {}     # (r,g,B) -> slot offset of cell within instr
    colmaps = []         # per instr: dict (k, B) -> cr (run-local col idx)
    wofs = 0
    c0 = 0
    arena_ch = 0
    ncrun_max = 0
    nci_max = 0
    for r, (b0, b1) in enumerate(runs):
        ch0 = 0
        cr_base = 0
        for g in range(NG):
            cc = caps[b0:b1, g]
            n = int(cc.sum())
            kb = (n + 127) // 128
            starts = np.zeros(b1 - b0 + 1, np.int64)
            np.cumsum(cc, out=starts[1:])
            colmap = {}
            ncols = 0
            # cols: for each chunk, each overlapping cell (ordered by B)
            for k in range(kb):
                lo, hi = k * 128, (k + 1) * 128
                i0 = int(np.searchsorted(starts, lo, "right") - 1)
                i1 = int(np.searchsorted(starts, hi, "left"))
                for bi in range(i0, min(i1, b1 - b0)):
                    colmap[(k, b0 + bi)] = ncols  # instr-local col index
                    p.block_mm[b0 + bi].append((r, cr_base + ncols, ch0 + k))
                    ncols += 1
            for bi in range(b1 - b0):
                p.cell_slot[(r, g, b0 + bi)] = int(starts[bi])
            q = (r + g) % 4
            p.instrs.append(dict(r=r, g=g, n=n, kb=kb, ch0=ch0,
                                 q=q, seq=qseq_ctr[q], wofs=wofs, c0=c0,
                                 ncols=ncols))
            qseq_ctr[q] += 1
            colmaps.append(colmap)
            wofs += n // 16
            c0 += ncols
            ch0 += kb
            cr_base += ncols
            nci_max = max(nci_max, ncols)
        arena_ch = max(arena_ch, ch0)
        ncrun_max = max(ncrun_max, cr_base)
    p.idx_w = wofs
    p.totcol = c0
    p.arena_ch = arena_ch
    p.ncrun = ncrun_max
    p.nci_max = nci_max
    p.caps = caps
    p.nruns = len(runs)

    p.run_instrs = [[] for _ in runs]
    for ins in p.instrs:
        p.run_instrs[ins["r"]].append(ins)

    # first/last flags per block
    p.block_sched = []
    for B in range(NB):
        mml = p.block_mm[B]
        p.block_sched.append(
            [(r, cr, ch, j == 0, j == len(mml) - 1) for j, (r, cr, ch) in enumerate(mml)]
        )

    # per-core data arrays
    idx_arrs, dv_arrs, deg_arrs = [], [], []
    for c in range(NCORES):
        src_c, dstin, cstarts = percore[c]
        idx16 = np.zeros((16, p.idx_w), np.int16)
        dv = np.full((128, p.totcol), -1.0, np.float32)
        for i, ins in enumerate(p.instrs):
            r, g = ins["r"], ins["g"]
            b0, b1 = runs[r]
            flat = np.zeros(ins["n"], np.int16)
            colmap = colmaps[i]
            for B in range(b0, b1):
                ci = B * NG + g
                cnt = int(counts[c, B, g])
                if cnt == 0:
                    continue
                e0 = int(cstarts[ci])
                s0 = p.cell_slot[(r, g, B)]
                flat[s0:s0 + cnt] = (src_c[e0:e0 + cnt] - g * GS).astype(np.int16)
                slots = s0 + np.arange(cnt)
                ks = slots >> 7
                ps = slots & 127
                gcs = np.array([colmap[(int(k), B)] for k in ks]) + ins["c0"]
                dv[ps, gcs] = (dstin[e0:e0 + cnt] - (B << 7)).astype(np.float32)
            w = ins["n"] // 16
            idx16[:, ins["wofs"]:ins["wofs"] + w] = flat.reshape(w, 16).T
        idx_arrs.append(np.tile(idx16, (8, 1)))
        dv_arrs.append(dv.astype(np.float32))

        deg = np.bincount(dstin, minlength=NB * 128).astype(np.float32)
        rdeg = 1.0 / np.maximum(deg, 1.0)
        ind = np.minimum(deg, 1.0)
        deg_arrs.append((rdeg.reshape(NB, 128).T.copy(), ind.reshape(NB, 128).T.copy()))
    return p, idx_arrs, dv_arrs, deg_arrs


def _build_nc(p):
    import concourse.bacc as bacc
    import concourse.mybir as mybir
    from concourse._compat import get_trn_type

    f32 = mybir.dt.float32
    bf16 = mybir.dt.bfloat16
    i16 = mybir.dt.int16

    nc = bacc.Bacc(get_trn_type() or "TRN2", debug=True, num_swdge_queues=4,
                   dynamic_dma_scratch_size=SCRATCH)
    h_d = nc.declare_dram_parameter("h", [N_NODES, 128], bf16, isOutput=False)
    idx_d = nc.declare_dram_parameter("idx", [128, p.idx_w], i16, isOutput=False)
    dv_d = nc.declare_dram_parameter("dv", [128, p.totcol], bf16, isOutput=False)
    iota_d = nc.declare_dram_parameter("iota", [128, 128 * p.nci_max], bf16, isOutput=False)
    cst_d = nc.declare_dram_parameter("cst", [128, 128], f32, isOutput=False)
    wa_d = nc.declare_dram_parameter("wa", [65, D], bf16, isOutput=False)
    rdeg_d = nc.declare_dram_parameter("rdeg", [128, NB], f32, isOutput=False)
    ind_d = nc.declare_dram_parameter("ind", [128, NB], f32, isOutput=False)
    out_d = nc.declare_dram_parameter("out", [D, NB * 128], f32, isOutput=True)

    NCR = p.ncrun
    ACH = p.arena_ch

    with ExitStack() as st:
        e = st.enter_context
        idx_sb = e(nc.sbuf_tensor("idx_sb", [128, p.idx_w], i16))
        dv_sb = e(nc.sbuf_tensor("dv_sb", [128, p.totcol], bf16))
        iota_sb = e(nc.sbuf_tensor("iota_sb", [128, 128 * p.nci_max], bf16))
        cst_sb = e(nc.sbuf_tensor("cst_sb", [128, 128], f32))
        wa_sb = e(nc.sbuf_tensor("wa_sb", [65, D], bf16))
        rdeg_sb = e(nc.sbuf_tensor("rdeg_sb", [128, NB], f32))
        ind_sb = e(nc.sbuf_tensor("ind_sb", [128, NB], f32))
        arena = e(nc.sbuf_tensor("arena", [128, ARENA_DEPTH * ACH * 128], bf16))
        sel_sb = e(nc.sbuf_tensor("sel_sb", [128, SEL_DEPTH * NCR * 128], bf16))
        agg = [e(nc.sbuf_tensor(f"agg{i}", [128, 65], f32)) for i in range(2)]
        aggT = [e(nc.sbuf_tensor(f"aggT{i}", [65, 128], bf16)) for i in range(2)]
        ysb = [e(nc.sbuf_tensor(f"ysb{i}", [64, 128], f32)) for i in range(2)]

        acc = [e(nc.psum_tensor(f"acc{i}", [128, 64], f32)) for i in range(2)]
        pt1 = [e(nc.psum_tensor(f"pt1_{i}", [65, 128], f32)) for i in range(2)]
        pmw = [e(nc.psum_tensor(f"pmw{i}", [64, 128], f32)) for i in range(2)]

        in_s = e(nc.semaphore("in_s"))
        g_q = [[e(nc.semaphore(f"g_q{q}_{d}")) for d in range(QDEPTH)]
               for q in range(4)]
        sel_s = e(nc.semaphore("sel_s"))
        blk_s = e(nc.semaphore("blk_s"))
        dep_s = e(nc.semaphore("dep_s"))
        pt1_s = e(nc.semaphore("pt1_s"))
        dt1_s = e(nc.semaphore("dt1_s"))
        pmw_s = e(nc.semaphore("pmw_s"))
        act_s = e(nc.semaphore("act_s"))
        out_p = [e(nc.semaphore(f"out_p{i}")) for i in range(2)]
        ms_s = e(nc.semaphore("ms_s"))
        block = e(nc.Block())

        def sel_view(r):
            base = (r % SEL_DEPTH) * NCR * 128
            return sel_sb[:, base:base + NCR * 128].rearrange(
                "p (d c) -> p d c", c=NCR)

        def arena_chunk(r, ch):
            base = ((r % ARENA_DEPTH) * ACH + ch) * 128
            return arena[:, base:base + 64]

        # run index of each block
        run_of = np.zeros(NB, np.int64)
        for r, (b0, b1) in enumerate(p.runs):
            run_of[b0:b1] = r

        @block.gpsimd
        def _(eng):
            eng.wait_ge(in_s, 112)  # all inputs loaded
            eng.wait_ge(ms_s, 1)    # arena zeroed
            for i, ins in enumerate(p.instrs):
                r = ins["r"]
                if ins["g"] == 0 and r >= ARENA_DEPTH:
                    eng.wait_ge(blk_s, p.runs[r - ARENA_DEPTH][1])
                sq = ins["seq"]
                if sq >= QDEPTH:
                    eng.wait_ge(g_q[ins["q"]][sq % QDEPTH], 16 * (sq // QDEPTH))
                base = ((r % ARENA_DEPTH) * ACH + ins["ch0"]) * 128
                eng.dma_gather(
                    out_ap=arena[:, base:base + ins["kb"] * 128].rearrange(
                        "p (k d) -> p k d", d=128),
                    in_ap=h_d[ins["g"] * GS:(ins["g"] + 1) * GS, :],
                    idxs_ap=idx_sb[:, ins["wofs"]:ins["wofs"] + ins["n"] // 16],
                    num_idxs=ins["n"],
                    num_idxs_reg=ins["n"],
                    elem_size=128,
                    queue_num=ins["q"],
                ).then_inc(g_q[ins["q"]][ins["seq"] % QDEPTH], 16)

        @block.vector
        def _(eng):
            import concourse.mybir as mb
            eng.memset(arena[:], 0.0).then_inc(ms_s, 1)
            eng.wait_ge(in_s, 112)

            def tail(B):
                pb = B % 2
                eng.wait_ge(blk_s, B + 1)
                eng.tensor_copy(out=agg[pb][:, 64:65], in_=ind_sb[:, B:B + 1])
                eng.tensor_scalar(
                    out=agg[pb][:, 0:64], in0=acc[pb][:, 0:64],
                    scalar1=rdeg_sb[:, B:B + 1], scalar2=None,
                    op0=mb.AluOpType.mult,
                ).then_inc(dep_s, 1)
                eng.wait_ge(pt1_s, B + 1)
                eng.tensor_copy(out=aggT[pb][:], in_=pt1[pb][:]).then_inc(dt1_s, 1)

            for r, (b0, b1) in enumerate(p.runs):
                if r >= SEL_DEPTH:
                    eng.wait_ge(blk_s, p.runs[r - SEL_DEPTH][1])
                sv = sel_view(r)
                cofs = 0
                for ins in p.run_instrs[r]:
                    ncols = ins["ncols"]
                    in0 = dv_sb[:, ins["c0"]:ins["c0"] + ncols].unsqueeze(1) \
                        .to_broadcast([128, 128, ncols])
                    in1 = iota_sb[:, :].rearrange(
                        "p (d c) -> p d c", c=p.nci_max)[:, :, 0:ncols]
                    eng.tensor_tensor(
                        out=sv[:, :, cofs:cofs + ncols],
                        in0=in0, in1=in1,
                        op=mb.AluOpType.is_equal,
                    ).then_inc(sel_s, 1)
                    cofs += ncols
                if r >= 1:
                    for B in range(*p.runs[r - 1]):
                        tail(B)
            for B in range(*p.runs[-1]):
                tail(B)

        @block.tensor
        def _(eng):
            eng.wait_ge(in_s, 112)

            def pe_ep(B):
                pb = B % 2
                eng.wait_ge(dep_s, B + 1)
                if B >= 2:
                    eng.wait_ge(dt1_s, B - 1)
                eng.matmul(
                    out=pt1[pb][:], lhsT=agg[pb][:], rhs=cst_sb[:, :],
                    is_transpose=True,
                ).then_inc(pt1_s, 1)
                eng.wait_ge(dt1_s, B + 1)
                if B >= 2:
                    eng.wait_ge(act_s, B - 1)
                eng.matmul(
                    out=pmw[pb][:], lhsT=wa_sb[:], rhs=aggT[pb][:],
                    start=True, stop=True,
                ).then_inc(pmw_s, 1)

            nsel = 0
            for r, (b0, b1) in enumerate(p.runs):
                for ins in p.run_instrs[r]:
                    sq = ins["seq"]
                    eng.wait_ge(g_q[ins["q"]][sq % QDEPTH],
                                16 * (sq // QDEPTH + 1))
                nsel += len(p.run_instrs[r])
                eng.wait_ge(sel_s, nsel)
                sv = sel_view(r)
                for B in range(b0, b1):
                    for (rr, cr, ch, first, last) in p.block_sched[B]:
                        assert rr == r
                        if first and B >= 2:
                            eng.wait_ge(dep_s, B - 1)
                        mm = eng.matmul(
                            out=acc[B % 2][:, 0:64],
                            lhsT=sv[:, :, cr],
                            rhs=arena_chunk(r, ch),
                            start=first, stop=last,
                        )
                        if last:
                            mm.then_inc(blk_s, 1)
                    if B >= 1:
                        pe_ep(B - 1)
            pe_ep(NB - 1)

        @block.scalar
        def _(eng):
            import concourse.mybir as mb
            for B in range(NB):
                eng.wait_ge(pmw_s, B + 1)
                if B >= 2:
                    eng.wait_ge(out_p[B % 2], 16 * (B // 2))
                eng.activation(
                    out=ysb[B % 2][:], in_=pmw[B % 2][:],
                    func=mb.ActivationFunctionType.Relu,
                ).then_inc(act_s, 1)

        @block.sync
        def _(eng):
            eng.dma_start(out=idx_sb[:], in_=idx_d[:]).then_inc(in_s, 16)
            eng.dma_start(out=dv_sb[:], in_=dv_d[:]).then_inc(in_s, 16)
            eng.dma_start(out=iota_sb[:], in_=iota_d[:]).then_inc(in_s, 16)
            eng.dma_start(out=cst_sb[:], in_=cst_d[:]).then_inc(in_s, 16)
            eng.dma_start(out=wa_sb[:], in_=wa_d[:]).then_inc(in_s, 16)
            eng.dma_start(out=rdeg_sb[:], in_=rdeg_d[:]).then_inc(in_s, 16)
            eng.dma_start(out=ind_sb[:], in_=ind_d[:]).then_inc(in_s, 16)
            for B in range(NB):
                eng.wait_ge(act_s, B + 1)
                eng.dma_start(
                    out=out_d[:, B * 128:(B + 1) * 128], in_=ysb[B % 2][:]
                ).then_inc(out_p[B % 2], 16)
            eng.wait_ge(out_p[0], 16 * ((NB + 1) // 2))
            eng.wait_ge(out_p[1], 16 * (NB // 2))

    nc.compile()
    return nc


def _host_inputs(h, W, b, p, idx_arrs, dv_arrs, deg_arrs):
    import ml_dtypes
    bf = ml_dtypes.bfloat16
    h_dual = np.zeros((N_NODES, 128), bf)
    hb = h.astype(bf)
    h_dual[:, 0:64] = hb
    h_dual[:, 64:128] = hb
    iota = np.zeros((128, 128 * p.nci_max), bf)
    iota[:] = np.repeat(np.arange(128, dtype=np.float32), p.nci_max)[None, :].astype(bf)
    cst = np.eye(128, dtype=np.float32)
    wa = np.concatenate([W.T.astype(np.float32), b.astype(np.float32)[None, :]],
                        axis=0).astype(bf)
    in_maps = []
    for c in range(NCORES):
        in_maps.append({
            "h": h_dual,
            "idx": idx_arrs[c],
            "dv": dv_arrs[c].astype(bf),
            "iota": iota,
            "cst": cst,
            "wa": wa,
            "rdeg": deg_arrs[c][0],
            "ind": deg_arrs[c][1],
        })
    return in_maps


def kernel(h, edge_src, edge_dst, W, b):
    h = np.asarray(h, np.float32)
    edge_src = np.asarray(edge_src, np.int32)
    edge_dst = np.asarray(edge_dst, np.int32)
    W = np.asarray(W, np.float32)
    b = np.asarray(b, np.float32)

    from concourse.bass_utils import run_bass_kernel_spmd

    p, idx_arrs, dv_arrs, deg_arrs = _host_plan(edge_src, edge_dst)
    nc = _build_nc(p)
    in_maps = _host_inputs(h, W, b, p, idx_arrs, dv_arrs, deg_arrs)
    res = run_bass_kernel_spmd(nc, in_maps, list(range(NCORES)))
    out = np.concatenate(
        [np.asarray(res.results[c]["out"])[:, :NPC].T for c in range(NCORES)], axis=0
    )
    return np.ascontiguousarray(out.astype(np.float32))
